# revision 46
# baseline (speedup 1.0000x reference)
import numpy as np
from contextlib import ExitStack

import concourse.bass as bass
import concourse.mybir as mybir
from concourse import library_config
from concourse.tile import TileContext
from concourse.tile_rust import add_dep_helper

F32 = mybir.dt.float32
F16 = mybir.dt.float16
I16 = mybir.dt.int16
I32 = mybir.dt.int32
U32 = mybir.dt.uint32
AF = mybir.ActivationFunctionType

B, N, DIM, K = 4, 4096, 128, 16
NQ = 2048
NT = 16
WCAND = 896
NSEG = WCAND // 128
NSLOT = NSEG * 8
NEG = -3.0e38


def bcast16(ap):
    return ap.rearrange("p q -> p q ()").to_broadcast(list(ap.shape) + [16])


def build(nc, stage="full", pwin=0.001, t0=12.0):
    xT_d = nc.dram_tensor("xT16", [DIM, N], F16, kind="ExternalInput")
    xTq_d = nc.dram_tensor("xTq16", [DIM, NQ], F16, kind="ExternalInput")
    posT_d = nc.dram_tensor("posT16r", [4, N], F16, kind="ExternalInput")
    posTq_d = nc.dram_tensor("posTq16r", [4, NQ], F16, kind="ExternalInput")
    qaugR_d = nc.dram_tensor("qaugR", [4, NQ], F32, kind="ExternalInput")
    caugW_d = nc.dram_tensor("caugW", [4, NT * WCAND], F32, kind="ExternalInput")
    candg_d = nc.dram_tensor("candg", [1, NT * WCAND], I32, kind="ExternalInput")
    kw_d = nc.dram_tensor("kw16", [DIM, DIM], F16, kind="ExternalInput")
    vw_d = nc.dram_tensor("vw16", [DIM, DIM], F16, kind="ExternalInput")
    pw1_d = nc.dram_tensor("pw1_16", [4, DIM], F16, kind="ExternalInput")
    pw2_d = nc.dram_tensor("pw2_16", [DIM, DIM], F16, kind="ExternalInput")
    wqa_d = nc.dram_tensor("wqa16", [DIM, 32], F16, kind="ExternalInput")
    aw1n_d = nc.dram_tensor("aw1n16", [DIM, 32], F16, kind="ExternalInput")
    wp_d = nc.dram_tensor("wp16", [DIM, 32], F16, kind="ExternalInput")
    aw2r_d = nc.dram_tensor("aw2rep", [64, DIM], F16, kind="ExternalInput")
    s16_d = nc.dram_tensor("s16hot", [DIM, NQ], F16, kind="ExternalInput")
    id16_d = nc.dram_tensor("id16", [DIM, DIM], F16, kind="ExternalInput")
    id16n_d = nc.dram_tensor("id16n", [DIM, DIM], F16, kind="ExternalInput")
    bh2_d = nc.dram_tensor("bias_h2x2", [64, 1], F32, kind="ExternalInput")
    bu_d = nc.dram_tensor("bias_u", [DIM, 1], F32, kind="ExternalInput")
    offs_d = nc.dram_tensor("offs", [DIM, NSLOT], U32, kind="ExternalInput")
    ranks_d = nc.dram_tensor("ranks", [DIM, 16], I16, kind="ExternalInput")

    tbl_d = nc.dram_tensor("tbl", [N, 3 * DIM], F16, kind="Internal")
    itmp_d = nc.dram_tensor("itmp", [NT, DIM, 16], I16, kind="Internal")
    itmp2_d = nc.dram_tensor("itmp2", [NT, 1, 2 * 2048], I16, kind="Internal")
    out_d = nc.dram_tensor("out", [DIM, NQ], F32, kind="ExternalOutput")

    with TileContext(nc) as tc, ExitStack() as ctx:
        const = ctx.enter_context(tc.tile_pool(name="const", bufs=1))
        work = ctx.enter_context(tc.tile_pool(name="work", bufs=2))
        gpool = ctx.enter_context(tc.tile_pool(name="gpool", bufs=3))
        apool = ctx.enter_context(tc.tile_pool(name="apool", bufs=2))
        spool = ctx.enter_context(tc.tile_pool(name="spool", bufs=2))
        ipool = ctx.enter_context(tc.tile_pool(name="ipool", bufs=4))
        cwpool = ctx.enter_context(tc.tile_pool(name="cwpool", bufs=3))
        agpool = ctx.enter_context(tc.tile_pool(name="agpool", bufs=2))
        psA = ctx.enter_context(tc.tile_pool(name="psA", bufs=4, space="PSUM"))
        psB = ctx.enter_context(tc.tile_pool(name="psB", bufs=4, space="PSUM"))

        def cload(d, shape, dtype):
            t = const.tile(shape, dtype, tag=d.name)
            nc.sync.dma_start(t, d[:, :])
            return t

        xT16 = cload(xT_d, [DIM, N], F16)
        kw = cload(kw_d, [DIM, DIM], F16)
        vw = cload(vw_d, [DIM, DIM], F16)
        pw2 = cload(pw2_d, [DIM, DIM], F16)
        wqa = cload(wqa_d, [DIM, 32], F16)
        aw1n = cload(aw1n_d, [DIM, 32], F16)
        wp = cload(wp_d, [DIM, 32], F16)
        aw2r = cload(aw2r_d, [64, DIM], F16)
        s16 = cload(s16_d, [DIM, NQ], F16)
        id16 = cload(id16_d, [DIM, DIM], F16)
        id16n = cload(id16n_d, [DIM, DIM], F16)
        bh2x2 = cload(bh2_d, [64, 1], F32)
        bu = cload(bu_d, [DIM, 1], F32)
        offs = cload(offs_d, [DIM, NSLOT], U32)
        ranks = cload(ranks_d, [DIM, 16], I16)
        xTq16 = cload(xTq_d, [DIM, NQ], F16)

        arena = const.tile([DIM, 3136], F32, tag="arena")
        pw1a = arena[0:4, 0:64].bitcast(F16)
        posTq16 = arena[0:4, 64:1088].bitcast(F16)
        posT16 = arena[0:4, 1088:3136].bitcast(F16)
        nc.sync.dma_start(pw1a, pw1_d[:, :])
        nc.sync.dma_start(posTq16, posTq_d[:, :])
        nc.sync.dma_start(posT16, posT_d[:, :])
        q_aug = const.tile([4, NQ], F32, tag="qaugt")
        nc.sync.dma_start(q_aug, qaugR_d[:, :])

        cgtabA = const.tile([16, WCAND], I32, tag="cgtabA")
        cgtabB = const.tile([16, WCAND], I32, tag="cgtabB")
        cgtabs = [cgtabA, cgtabB]
        nc.gpsimd.memset(cgtabA, 0)
        nc.gpsimd.memset(cgtabB, 0)

        for blk in range(N // DIM):
            bsl = slice(blk * DIM, (blk + 1) * DIM)
            ps = psB.tile([DIM, 512], F32, tag="mm", name="ps")
            ps = ps[:, 0:3 * DIM]
            nc.tensor.matmul(ps[:, 0:128], lhsT=xT16[:, bsl], rhs=kw,
                             start=True, stop=True)
            nc.tensor.matmul(ps[:, 128:256], lhsT=xT16[:, bsl], rhs=vw,
                             start=True, stop=True)
            nc.tensor.matmul(ps[:, 256:384], lhsT=posT16[:, bsl], rhs=pw1a,
                             start=True, stop=True)
            tb = work.tile([DIM, 3 * DIM], F16, tag="ldx")
            nc.scalar.copy(tb, ps)
            nc.sync.dma_start(tbl_d[bsl, :], tb)

        if stage == "table":
            return nc

        idxall = const.tile([DIM, NT * DIM], I16, tag="idxall")
        pending_pool = []

        def lib_load(lib):
            ll = nc.gpsimd.load_library(lib)
            for op in pending_pool:
                add_dep_helper(ll.ins, op.ins, reason="lib window order")
            del pending_pool[:]
            return ll

        cw_tiles, sel_tiles, idxl_tiles = {}, {}, {}

        def emit_cw_prefetch(t):
            cw = cwpool.tile([4, WCAND], F32, tag="cw")
            nc.sync.dma_start(cw, caugW_d[:, t * WCAND:(t + 1) * WCAND])
            cw_tiles[t] = cw

        def emit_sel_scan(t):
            qsl = slice(t * DIM, (t + 1) * DIM)
            cw = cw_tiles.pop(t)
            v8s = ipool.tile([DIM, NSLOT], F32, tag="v8s")
            p8s = ipool.tile([DIM, NSLOT], U32, tag="p8s")
            ch = 0
            for bk, wd in enumerate([512, WCAND - 512]):
                dps = psA.tile([DIM, 512], F32, tag="dps", name="dps")
                dps = dps[:, 0:wd]
                nc.tensor.matmul(dps, lhsT=q_aug[:, qsl],
                                 rhs=cw[:, bk * 512:bk * 512 + wd],
                                 start=True, stop=True)
                for sg in range(wd // 128):
                    s8 = slice(ch * 8, (ch + 1) * 8)
                    seg = dps[:, sg * 128:(sg + 1) * 128]
                    nc.vector.max(out=v8s[:, s8], in_=seg)
                    nc.vector.max_index(out=p8s[:, s8], in_max=v8s[:, s8],
                                        in_values=seg)
                    ch += 1
            posg16 = ipool.tile([DIM, NSLOT], I16, tag="posg16")
            nc.vector.tensor_add(posg16, p8s, offs)

            mm8 = ipool.tile([DIM, 16], F32, tag="mm8")
            m8a, m8b = mm8[:, 0:8], mm8[:, 8:16]
            sl16 = ipool.tile([DIM, 16], U32, tag="sl16")
            nc.vector.max(out=m8a, in_=v8s)
            nc.vector.max_index(out=sl16[:, 0:8], in_max=m8a, in_values=v8s)
            nc.vector.match_replace(out=v8s, in_to_replace=m8a, in_values=v8s,
                                    imm_value=NEG)
            nc.vector.max(out=m8b, in_=v8s)
            nc.vector.max_index(out=sl16[:, 8:16], in_max=m8b, in_values=v8s)
            slots16 = ipool.tile([DIM, 16], I16, tag="slots16")
            nc.vector.tensor_copy(slots16, sl16)
            sel_tiles[t] = (slots16, posg16)

        def emit_sel_finish(t, llA):
            slots16, posg16 = sel_tiles.pop(t)
            R = ipool.tile([DIM, NSLOT], I16, tag="R")
            ls1 = nc.gpsimd.local_scatter(R[:, :], ranks[:, :], slots16[:, :],
                                          channels=DIM, num_elems=NSLOT,
                                          num_idxs=16)
            add_dep_helper(ls1.ins, llA.ins, reason="needs local_scatter lib")
            pending_pool.append(ls1)
            nc.vector.tensor_scalar(R, R, 1, None,
                                    op0=mybir.AluOpType.subtract)
            idxsel = ipool.tile([DIM, 16], I16, tag="idxsel")
            ls2 = nc.gpsimd.local_scatter(idxsel[:, :], posg16[:, :], R[:, :],
                                          channels=DIM, num_elems=16,
                                          num_idxs=NSLOT)
            add_dep_helper(ls2.ins, llA.ins, reason="needs local_scatter lib")
            pending_pool.append(ls2)

            nc.sync.dma_start(itmp_d[t, :, :], idxsel)
            idxl = ipool.tile([16, DIM], I16, tag="idxl")
            nc.sync.dma_start(idxl, itmp_d[t, :, :].rearrange("a b -> b a"))
            idxl_tiles[t] = idxl
            nc.sync.dma_start(cgtabs[t % 2][0:1, :],
                              candg_d[:, t * WCAND:(t + 1) * WCAND])

        def emit_translate(t, llC):
            isl = slice(t * DIM, (t + 1) * DIM)
            idxl = idxl_tiles.pop(t)
            ag = agpool.tile([16, 2048], I32, tag="ag")
            gi = nc.gpsimd.ap_gather(out_ap=ag[:, :],
                                     in_ap=cgtabs[t % 2][:, :],
                                     idxs_ap=idxl[:, :], channels=16,
                                     num_elems=WCAND, d=1, num_idxs=2048)
            add_dep_helper(gi.ins, llC.ins, reason="needs ap_gather lib")
            pending_pool.append(gi)
            nc.sync.dma_start(itmp2_d[t, :, :], ag[0:1, :].bitcast(I16))
            src = itmp2_d[t, 0, :].rearrange("(q k two) -> k q two",
                                             k=16, two=2)[:, :, 0:1]
            nc.sync.dma_start(
                idxall[0:16, isl].rearrange("k q -> k q ()"), src)
            nc.sync.dma_start(idxall[16:32, isl], idxall[0:16, isl])
            nc.sync.dma_start(idxall[32:64, isl], idxall[0:32, isl])
            nc.sync.dma_start(idxall[64:128, isl], idxall[0:64, isl])

        g_tiles, ev_tiles = {}, {}

        def emit_gather(t, llB):
            ibase = t * DIM
            g = gpool.tile([DIM, 3, 4, 512], F16, tag="g")
            g_tiles[t] = g
            for gc in range(4):
                gi = nc.gpsimd.dma_gather(
                    out_ap=g[:, :, gc, :],
                    in_ap=tbl_d[:, :],
                    idxs_ap=idxall[:, ibase + gc * 32:ibase + (gc + 1) * 32],
                    num_idxs=512,
                    num_idxs_reg=512,
                    elem_size=3 * DIM,
                    transpose=True,
                )
                add_dep_helper(gi.ins, llB.ins, reason="needs mlp lib")
                pending_pool.append(gi)

        def emit_attn_mlp(t):
            qsl = slice(t * DIM, (t + 1) * DIM)
            g = g_tiles[t]

            qp16 = spool.tile([DIM, DIM + 32], F16, tag="qp16")
            xwqa, p1qM = qp16[:, 0:32], qp16[:, 32:DIM + 32]
            qps = psB.tile([DIM, 512], F32, tag="mm", name="qps")
            qps = qps[:, 0:DIM + 32]
            nc.tensor.matmul(qps[:, 0:32], lhsT=xTq16[:, qsl], rhs=wqa,
                             start=True, stop=True)
            nc.tensor.matmul(qps[:, 32:32 + DIM], lhsT=posTq16[:, qsl],
                             rhs=pw1a, start=True, stop=True)
            nc.scalar.copy(qp16, qps)

            hid = apool.tile([DIM, NQ], F16, tag="hid")
            h2st = spool.tile([64, NQ // 2], F16, tag="h2st")
            we = apool.tile([DIM, 2 * NQ], F16, tag="we", bufs=3)
            e = we[:, NQ:2 * NQ]
            vpe = apool.tile([DIM, NQ], F16, tag="sp", bufs=3)
            for gc in range(4):
                ssl = slice(gc * 512, (gc + 1) * 512)
                sps = psB.tile([DIM, 512], F32, tag="mm", name="sps")
                nc.tensor.matmul(sps, lhsT=p1qM, rhs=s16[:, ssl],
                                 start=True, stop=False)
                nc.tensor.matmul(sps, lhsT=id16n, rhs=g[:, 2, ssl],
                                 start=False, stop=True)
                nc.scalar.activation(hid[:, ssl], sps, AF.Relu)
            for pr in range(2):
                hp = psB.tile([DIM, 512], F32, tag="mm", name="hp")
                for gc in range(2 * pr, 2 * pr + 2):
                    ssl = slice(gc * 512, (gc + 1) * 512)
                    hpc = hp[(gc % 2) * 32:(gc % 2) * 32 + 32, :]
                    nc.tensor.matmul(hpc, lhsT=xwqa, rhs=s16[:, ssl],
                                     start=True, stop=False)
                    nc.tensor.matmul(hpc, lhsT=aw1n, rhs=g[:, 0, ssl],
                                     start=False, stop=False)
                    nc.tensor.matmul(hpc, lhsT=wp, rhs=hid[:, ssl],
                                     start=False, stop=True)
                nc.scalar.activation(h2st[:, pr * 512:(pr + 1) * 512],
                                     hp[0:64, :], AF.Relu, bias=bh2x2)
            for gc in range(4):
                ssl = slice(gc * 512, (gc + 1) * 512)
                vp = psB.tile([DIM, 512], F32, tag="mm", name="vp")
                nc.tensor.matmul(vp, lhsT=pw2, rhs=hid[:, ssl],
                                 start=True, stop=False)
                nc.tensor.matmul(vp, lhsT=id16, rhs=g[:, 1, ssl],
                                 start=False, stop=True)
                nc.scalar.activation(vpe[:, ssl], vp, AF.Identity)
            for gc in range(4):
                ssl = slice(gc * 512, (gc + 1) * 512)
                lp = psB.tile([DIM, 512], F32, tag="mm", name="lp")
                h2c = h2st[(gc % 2) * 32:(gc % 2) * 32 + 32,
                           (gc // 2) * 512:(gc // 2) * 512 + 512]
                nc.tensor.matmul(lp,
                                 lhsT=aw2r[(gc % 2) * 32:(gc % 2) * 32 + 32, :],
                                 rhs=h2c, start=True, stop=True)
                nc.scalar.activation(e[:, ssl], lp, AF.Exp)
            ev_tiles[t] = (we, vpe)

        def emit_attn_red(t):
            qsl = slice(t * DIM, (t + 1) * DIM)
            we, vpe = ev_tiles.pop(t)
            del g_tiles[t]
            quad = spool.tile([DIM, 512], F32, tag="quad")
            ws, es = quad[:, 0:128], quad[:, 128:256]
            rec, ot = quad[:, 256:384], quad[:, 384:512]
            nc.vector.tensor_mul(we[:, 0:NQ], we[:, NQ:2 * NQ], vpe)

            fs = apool.tile([DIM, 3584], F16, tag="fold")
            s3 = we.rearrange("p (q k) -> p q k", k=16)
            L1 = fs[:, 0:2048].rearrange("p (q k) -> p q k", k=8)
            nc.vector.tensor_add(L1, s3[:, :, 0:8], s3[:, :, 8:16])
            L1v = fs[:, 0:2048].rearrange("p (q k) -> p q k", k=8)
            L2 = fs[:, 2048:3072].rearrange("p (q k) -> p q k", k=4)
            nc.vector.tensor_add(L2, L1v[:, :, 0:4], L1v[:, :, 4:8])
            L2v = fs[:, 2048:3072].rearrange("p (q k) -> p q k", k=4)
            L3 = fs[:, 3072:3584].rearrange("p (q k) -> p q k", k=2)
            nc.vector.tensor_add(L3, L2v[:, :, 0:2], L2v[:, :, 2:4])
            L3v = fs[:, 3072:3584].rearrange("p (q k) -> p q k", k=2)
            nc.vector.tensor_add(quad[:, 0:256].rearrange("p q -> p q ()"),
                                 L3v[:, :, 0:1], L3v[:, :, 1:2])
            nc.vector.scalar_tensor_tensor(ws, es, bu, ws,
                                           op0=mybir.AluOpType.mult,
                                           op1=mybir.AluOpType.add)
            nc.vector.reciprocal(rec, es)
            nc.vector.tensor_mul(ot, ws, rec)

            nc.sync.dma_start(out_d[:, qsl], ot)

        for w in range(-3, NT + 2):
            with tc.tile_wait_until((t0 + (w + 3) * pwin) / 1000.0):
                if 0 <= w + 3 < NT:
                    emit_cw_prefetch(w + 3)
                if 0 <= w + 2 < NT:
                    emit_sel_scan(w + 2)
                if 0 <= w - 1 < NT:
                    emit_attn_mlp(w - 1)
                if 0 <= w + 1 < NT:
                    llC = lib_load(library_config.ap_gather)
                    emit_translate(w + 1, llC)
                if 0 <= w + 2 < NT:
                    llA = lib_load(library_config.local_scatter)
                    emit_sel_finish(w + 2, llA)
                if 0 <= w < NT:
                    llB = lib_load(library_config.mlp)
                    emit_gather(w, llB)
                if 0 <= w - 2 < NT:
                    emit_attn_red(w - 2)
    return nc



def kd_order(p, leaf=128):
    out = []

    def rec(ids):
        if len(ids) <= leaf:
            out.append(ids)
            return
        ext = p[ids].max(0) - p[ids].min(0)
        ax = int(np.argmax(ext))
        srt = ids[np.argsort(p[ids, ax], kind="stable")]
        h = len(srt) // 2
        rec(srt[:h])
        rec(srt[h:])

    rec(np.arange(len(p)))
    return np.concatenate(out)


def make_in_maps(inputs):
    x, pos = np.asarray(inputs["x"]), np.asarray(inputs["pos"])
    f16 = np.float16
    W = {k: np.asarray(v, np.float32) for k, v in inputs.items()}
    pw1p = np.zeros((4, DIM), np.float32)
    pw1p[:3] = W["pw1"]
    pw1p[3] = -W["pb1"]
    s_hot = np.zeros((DIM, NQ), f16)
    s_hot[np.repeat(np.arange(DIM), 16), np.arange(NQ)] = 1.0
    aw2rep = np.zeros((64, DIM), f16)
    for g_ in range(2):
        aw2rep[32 * g_:32 * g_ + 32] = W["aw2"].astype(f16)
    bh2 = (W["ab1"] + (W["qb"] - W["kb"] + W["pb2"]) @ W["aw1"]).reshape(32, 1)
    shared = {
        "kw16": np.ascontiguousarray(W["kw"].astype(f16)),
        "vw16": np.ascontiguousarray(W["vw"].astype(f16)),
        "pw1_16": np.ascontiguousarray(pw1p.astype(f16)),
        "pw2_16": np.ascontiguousarray(W["pw2"].astype(f16)),
        "wqa16": np.ascontiguousarray((W["qw"] @ W["aw1"]).astype(f16)),
        "aw1n16": np.ascontiguousarray((-W["aw1"]).astype(f16)),
        "wp16": np.ascontiguousarray((W["pw2"] @ W["aw1"]).astype(f16)),
        "aw2rep": np.ascontiguousarray(aw2rep),
        "s16hot": np.ascontiguousarray(s_hot),
        "id16": np.eye(DIM, dtype=f16),
        "id16n": (-np.eye(DIM)).astype(f16),
        "bias_h2x2": np.ascontiguousarray(np.tile(bh2, (2, 1)).astype(np.float32)),
        "bias_u": np.ascontiguousarray((W["vb"] + W["pb2"]).reshape(DIM, 1)),
        "offs": np.broadcast_to((np.arange(NSLOT, dtype=np.uint32) // 8) * 128,
                                (DIM, NSLOT)).copy(),
        "ranks": np.broadcast_to(np.arange(1, 17, dtype=np.int16),
                                 (DIM, 16)).copy(),
    }
    orders = []
    in_maps = []
    for b in range(B):
        orders.append(kd_order(pos[b].astype(np.float64)))
    for c in range(8):
        b, h = c // 2, c % 2
        order = orders[b]
        ps = pos[b].astype(np.float64)[order]
        xs = np.asarray(x[b], np.float32)[order]
        qs = slice(h * NQ, (h + 1) * NQ)

        p32 = ps.astype(np.float32)
        caug = np.zeros((N, 4), np.float32)
        caug[:, :3] = 2.0 * p32
        caug[:, 3] = (p32[:, 0] * p32[:, 0] + p32[:, 1] * p32[:, 1]) \
            + p32[:, 2] * p32[:, 2]
        qaug = np.zeros((N, 4), np.float32)
        qaug[:, :3] = p32
        qaug[:, 3] = -1.0
        post = np.zeros((4, N), f16)
        post[:3] = p32.T.astype(f16)
        postq = np.zeros((4, NQ), f16)
        postq[:3] = p32[qs].T.astype(f16)
        postq[3] = -1.0

        caugW = np.zeros((4, NT * WCAND), np.float32)
        candg = np.zeros((1, NT * WCAND), np.int32)
        rng = np.random.default_rng(97 + c)
        for t in range(NT):
            rows = slice(h * NQ + t * DIM, h * NQ + (t + 1) * DIM)
            tmin, tmax = ps[rows].min(0), ps[rows].max(0)
            gap = np.maximum(0, np.maximum(tmin[None] - ps, ps - tmax[None]))
            cand = np.argsort((gap ** 2).sum(-1), kind="stable")[:WCAND]
            cand = cand[rng.permutation(WCAND)]
            caugW[:, t * WCAND:(t + 1) * WCAND] = caug[cand].T
            candg[0, t * WCAND:(t + 1) * WCAND] = cand
        m = dict(shared)
        m["xT16"] = np.ascontiguousarray(xs.T.astype(f16))
        m["xTq16"] = np.ascontiguousarray(xs[qs].T.astype(f16))
        m["posT16r"] = np.ascontiguousarray(post)
        m["posTq16r"] = np.ascontiguousarray(postq)
        m["qaugR"] = np.ascontiguousarray(qaug[qs].T)
        m["caugW"] = caugW
        m["candg"] = candg
        in_maps.append(m)
    return in_maps, orders


_CACHED = {}


def run(inputs, trace=False, **spmd_kwargs):
    from concourse.bass_utils import run_bass_kernel_spmd

    if "nc" not in _CACHED:
        import concourse.bacc as bacc
        nc = bacc.Bacc("TRN2", target_bir_lowering=False, debug=False,
                       num_devices=8)
        build(nc)
        nc.compile()
        _CACHED["nc"] = nc
    nc = _CACHED["nc"]
    in_maps, orders = make_in_maps(inputs)
    res = run_bass_kernel_spmd(nc, in_maps, core_ids=list(range(8)),
                               trace=trace, **spmd_kwargs)
    out = np.empty((B, N, DIM), np.float32)
    for c in range(8):
        b, h = c // 2, c % 2
        rows = orders[b][h * NQ:(h + 1) * NQ]
        out[b, rows] = res.results[c]["out"].T
    return out, res


def kernel(**inputs):
    return run(inputs)[0]


# revision 47
# speedup vs baseline: 1.0244x; 1.0244x over previous
import numpy as np
from contextlib import ExitStack

import concourse.bass as bass
import concourse.mybir as mybir
from concourse import library_config
from concourse.tile import TileContext
from concourse.tile_rust import add_dep_helper

F32 = mybir.dt.float32
F16 = mybir.dt.float16
I16 = mybir.dt.int16
I32 = mybir.dt.int32
U32 = mybir.dt.uint32
AF = mybir.ActivationFunctionType

B, N, DIM, K = 4, 4096, 128, 16
NQ = 2048
NT = 16
WCAND = 896
NSEG = WCAND // 128
NSLOT = NSEG * 8
NEG = -3.0e38


def bcast16(ap):
    return ap.rearrange("p q -> p q ()").to_broadcast(list(ap.shape) + [16])


def build(nc, stage="full", pwin=0.001, t0=12.0):
    xT_d = nc.dram_tensor("xT16", [DIM, N], F16, kind="ExternalInput")
    xTq_d = nc.dram_tensor("xTq16", [DIM, NQ], F16, kind="ExternalInput")
    posT_d = nc.dram_tensor("posT16r", [4, N], F16, kind="ExternalInput")
    posTq_d = nc.dram_tensor("posTq16r", [4, NQ], F16, kind="ExternalInput")
    qaugR_d = nc.dram_tensor("qaugR", [4, NQ], F32, kind="ExternalInput")
    caugW_d = nc.dram_tensor("caugW", [4, NT * WCAND], F32, kind="ExternalInput")
    candg_d = nc.dram_tensor("candg", [1, NT * WCAND], I32, kind="ExternalInput")
    kw_d = nc.dram_tensor("kw16", [DIM, DIM], F16, kind="ExternalInput")
    vw_d = nc.dram_tensor("vw16", [DIM, DIM], F16, kind="ExternalInput")
    pw1_d = nc.dram_tensor("pw1_16", [4, DIM], F16, kind="ExternalInput")
    pw2_d = nc.dram_tensor("pw2_16", [DIM, DIM], F16, kind="ExternalInput")
    wqa_d = nc.dram_tensor("wqa16", [DIM, 32], F16, kind="ExternalInput")
    aw1n_d = nc.dram_tensor("aw1n16", [DIM, 32], F16, kind="ExternalInput")
    wp_d = nc.dram_tensor("wp16", [DIM, 32], F16, kind="ExternalInput")
    aw2r_d = nc.dram_tensor("aw2rep", [64, DIM], F16, kind="ExternalInput")
    s16_d = nc.dram_tensor("s16hot", [DIM, NQ], F16, kind="ExternalInput")
    id16_d = nc.dram_tensor("id16", [DIM, DIM], F16, kind="ExternalInput")
    id16n_d = nc.dram_tensor("id16n", [DIM, DIM], F16, kind="ExternalInput")
    bh2_d = nc.dram_tensor("bias_h2x2", [64, 1], F32, kind="ExternalInput")
    bu_d = nc.dram_tensor("bias_u", [DIM, 1], F32, kind="ExternalInput")
    offs_d = nc.dram_tensor("offs", [DIM, NSLOT], U32, kind="ExternalInput")
    ranks_d = nc.dram_tensor("ranks", [DIM, 16], I16, kind="ExternalInput")
    pw1n4_d = nc.dram_tensor("pw1n4_16", [4, DIM], F16, kind="ExternalInput")
    id32n_d = nc.dram_tensor("id32n128", [DIM, 32], F16, kind="ExternalInput")
    idp_d = nc.dram_tensor("idp128_16", [4, DIM], F16, kind="ExternalInput")
    kwa_d = nc.dram_tensor("kwa32_16", [DIM, 32], F16, kind="ExternalInput")

    tbl_d = nc.dram_tensor("tbl", [N, 2 * DIM], F16, kind="Internal")
    itmp_d = nc.dram_tensor("itmp", [NT, DIM, 16], I16, kind="Internal")
    itmp2_d = nc.dram_tensor("itmp2", [NT, 1, 2 * 2048], I16, kind="Internal")
    out_d = nc.dram_tensor("out", [DIM, NQ], F32, kind="ExternalOutput")

    with TileContext(nc) as tc, ExitStack() as ctx:
        const = ctx.enter_context(tc.tile_pool(name="const", bufs=1))
        work = ctx.enter_context(tc.tile_pool(name="work", bufs=2))
        gpool = ctx.enter_context(tc.tile_pool(name="gpool", bufs=3))
        apool = ctx.enter_context(tc.tile_pool(name="apool", bufs=2))
        spool = ctx.enter_context(tc.tile_pool(name="spool", bufs=2))
        ipool = ctx.enter_context(tc.tile_pool(name="ipool", bufs=4))
        cwpool = ctx.enter_context(tc.tile_pool(name="cwpool", bufs=3))
        agpool = ctx.enter_context(tc.tile_pool(name="agpool", bufs=2))
        psA = ctx.enter_context(tc.tile_pool(name="psA", bufs=4, space="PSUM"))
        psB = ctx.enter_context(tc.tile_pool(name="psB", bufs=4, space="PSUM"))

        def cload(d, shape, dtype):
            t = const.tile(shape, dtype, tag=d.name)
            nc.sync.dma_start(t, d[:, :])
            return t

        xT16 = cload(xT_d, [DIM, N], F16)
        kw = cload(kw_d, [DIM, DIM], F16)
        vw = cload(vw_d, [DIM, DIM], F16)
        pw2 = cload(pw2_d, [DIM, DIM], F16)
        wqa = cload(wqa_d, [DIM, 32], F16)
        aw1n = cload(aw1n_d, [DIM, 32], F16)
        wp = cload(wp_d, [DIM, 32], F16)
        aw2r = cload(aw2r_d, [64, DIM], F16)
        s16 = cload(s16_d, [DIM, NQ], F16)
        id16 = cload(id16_d, [DIM, DIM], F16)
        id16n = cload(id16n_d, [DIM, DIM], F16)
        bh2x2 = cload(bh2_d, [64, 1], F32)
        bu = cload(bu_d, [DIM, 1], F32)
        offs = cload(offs_d, [DIM, NSLOT], U32)
        ranks = cload(ranks_d, [DIM, 16], I16)
        pw1n4 = cload(pw1n4_d, [4, DIM], F16)
        id32n128 = cload(id32n_d, [DIM, 32], F16)
        idp128 = cload(idp_d, [4, DIM], F16)
        kwa32 = cload(kwa_d, [DIM, 32], F16)
        xTq16 = cload(xTq_d, [DIM, NQ], F16)

        arena = const.tile([DIM, 3136], F32, tag="arena")
        pw1a = arena[0:4, 0:64].bitcast(F16)
        posTq16 = arena[0:4, 64:1088].bitcast(F16)
        posT16 = arena[0:4, 1088:3136].bitcast(F16)
        nc.sync.dma_start(pw1a, pw1_d[:, :])
        nc.sync.dma_start(posTq16, posTq_d[:, :])
        nc.sync.dma_start(posT16, posT_d[:, :])
        q_aug = const.tile([4, NQ], F32, tag="qaugt")
        nc.sync.dma_start(q_aug, qaugR_d[:, :])

        cgtabA = const.tile([16, WCAND], I32, tag="cgtabA")
        cgtabB = const.tile([16, WCAND], I32, tag="cgtabB")
        cgtabs = [cgtabA, cgtabB]
        nc.gpsimd.memset(cgtabA, 0)
        nc.gpsimd.memset(cgtabB, 0)

        for blk in range(N // DIM):
            bsl = slice(blk * DIM, (blk + 1) * DIM)
            ps = psB.tile([DIM, 512], F32, tag="mm", name="ps")
            ps = ps[:, 0:2 * DIM]
            nc.tensor.matmul(ps[:, 0:128], lhsT=posT16[:, bsl], rhs=idp128,
                             start=True, stop=True)
            nc.tensor.matmul(ps[:, 32:64], lhsT=xT16[:, bsl], rhs=kwa32,
                             start=True, stop=True)
            nc.tensor.matmul(ps[:, 128:256], lhsT=xT16[:, bsl], rhs=vw,
                             start=True, stop=True)
            tb = work.tile([DIM, 2 * DIM], F16, tag="ldx")
            nc.scalar.copy(tb, ps)
            nc.sync.dma_start(tbl_d[bsl, :], tb)

        if stage == "table":
            return nc

        idxall = const.tile([DIM, NT * DIM], I16, tag="idxall")
        pending_pool = []

        def lib_load(lib):
            ll = nc.gpsimd.load_library(lib)
            for op in pending_pool:
                add_dep_helper(ll.ins, op.ins, reason="lib window order")
            del pending_pool[:]
            return ll

        cw_tiles, sel_tiles, idxl_tiles = {}, {}, {}

        def emit_cw_prefetch(t):
            cw = cwpool.tile([4, WCAND], F32, tag="cw")
            nc.sync.dma_start(cw, caugW_d[:, t * WCAND:(t + 1) * WCAND])
            cw_tiles[t] = cw

        def emit_sel_scan(t):
            qsl = slice(t * DIM, (t + 1) * DIM)
            cw = cw_tiles.pop(t)
            v8s = ipool.tile([DIM, NSLOT], F32, tag="v8s")
            p8s = ipool.tile([DIM, NSLOT], U32, tag="p8s")
            ch = 0
            for bk, wd in enumerate([512, WCAND - 512]):
                dps = psA.tile([DIM, 512], F32, tag="dps", name="dps")
                dps = dps[:, 0:wd]
                nc.tensor.matmul(dps, lhsT=q_aug[:, qsl],
                                 rhs=cw[:, bk * 512:bk * 512 + wd],
                                 start=True, stop=True)
                for sg in range(wd // 128):
                    s8 = slice(ch * 8, (ch + 1) * 8)
                    seg = dps[:, sg * 128:(sg + 1) * 128]
                    nc.vector.max(out=v8s[:, s8], in_=seg)
                    nc.vector.max_index(out=p8s[:, s8], in_max=v8s[:, s8],
                                        in_values=seg)
                    ch += 1
            posg16 = ipool.tile([DIM, NSLOT], I16, tag="posg16")
            nc.vector.tensor_add(posg16, p8s, offs)

            mm8 = ipool.tile([DIM, 16], F32, tag="mm8")
            m8a, m8b = mm8[:, 0:8], mm8[:, 8:16]
            sl16 = ipool.tile([DIM, 16], U32, tag="sl16")
            nc.vector.max(out=m8a, in_=v8s)
            nc.vector.max_index(out=sl16[:, 0:8], in_max=m8a, in_values=v8s)
            nc.vector.match_replace(out=v8s, in_to_replace=m8a, in_values=v8s,
                                    imm_value=NEG)
            nc.vector.max(out=m8b, in_=v8s)
            nc.vector.max_index(out=sl16[:, 8:16], in_max=m8b, in_values=v8s)
            slots16 = ipool.tile([DIM, 16], I16, tag="slots16")
            nc.vector.tensor_copy(slots16, sl16)
            sel_tiles[t] = (slots16, posg16)

        def emit_sel_finish(t, llA):
            slots16, posg16 = sel_tiles.pop(t)
            R = ipool.tile([DIM, NSLOT], I16, tag="R")
            ls1 = nc.gpsimd.local_scatter(R[:, :], ranks[:, :], slots16[:, :],
                                          channels=DIM, num_elems=NSLOT,
                                          num_idxs=16)
            add_dep_helper(ls1.ins, llA.ins, reason="needs local_scatter lib")
            pending_pool.append(ls1)
            nc.vector.tensor_scalar(R, R, 1, None,
                                    op0=mybir.AluOpType.subtract)
            idxsel = ipool.tile([DIM, 16], I16, tag="idxsel")
            ls2 = nc.gpsimd.local_scatter(idxsel[:, :], posg16[:, :], R[:, :],
                                          channels=DIM, num_elems=16,
                                          num_idxs=NSLOT)
            add_dep_helper(ls2.ins, llA.ins, reason="needs local_scatter lib")
            pending_pool.append(ls2)

            nc.sync.dma_start(itmp_d[t, :, :], idxsel)
            idxl = ipool.tile([16, DIM], I16, tag="idxl")
            nc.sync.dma_start(idxl, itmp_d[t, :, :].rearrange("a b -> b a"))
            idxl_tiles[t] = idxl
            nc.sync.dma_start(cgtabs[t % 2][0:1, :],
                              candg_d[:, t * WCAND:(t + 1) * WCAND])

        def emit_translate(t, llC):
            isl = slice(t * DIM, (t + 1) * DIM)
            idxl = idxl_tiles.pop(t)
            ag = agpool.tile([16, 2048], I32, tag="ag")
            gi = nc.gpsimd.ap_gather(out_ap=ag[:, :],
                                     in_ap=cgtabs[t % 2][:, :],
                                     idxs_ap=idxl[:, :], channels=16,
                                     num_elems=WCAND, d=1, num_idxs=2048)
            add_dep_helper(gi.ins, llC.ins, reason="needs ap_gather lib")
            pending_pool.append(gi)
            nc.sync.dma_start(itmp2_d[t, :, :], ag[0:1, :].bitcast(I16))
            src = itmp2_d[t, 0, :].rearrange("(q k two) -> k q two",
                                             k=16, two=2)[:, :, 0:1]
            nc.sync.dma_start(
                idxall[0:16, isl].rearrange("k q -> k q ()"), src)
            nc.sync.dma_start(idxall[16:32, isl], idxall[0:16, isl])
            nc.sync.dma_start(idxall[32:64, isl], idxall[0:32, isl])
            nc.sync.dma_start(idxall[64:128, isl], idxall[0:64, isl])

        g_tiles, ev_tiles = {}, {}

        def emit_gather(t, llB):
            ibase = t * DIM
            g = gpool.tile([DIM, 3, 4, 512], F16, tag="g")
            g_tiles[t] = g
            for gc in range(4):
                gi = nc.gpsimd.dma_gather(
                    out_ap=g[:, :, gc, :],
                    in_ap=tbl_d[:, :],
                    idxs_ap=idxall[:, ibase + gc * 32:ibase + (gc + 1) * 32],
                    num_idxs=512,
                    num_idxs_reg=512,
                    elem_size=2 * DIM,
                    transpose=True,
                )
                add_dep_helper(gi.ins, llB.ins, reason="needs mlp lib")
                pending_pool.append(gi)

        def emit_attn_mlp(t):
            qsl = slice(t * DIM, (t + 1) * DIM)
            g = g_tiles[t]

            qp16 = spool.tile([DIM, DIM + 32], F16, tag="qp16")
            xwqa, p1qM = qp16[:, 0:32], qp16[:, 32:DIM + 32]
            qps = psB.tile([DIM, 512], F32, tag="mm", name="qps")
            qps = qps[:, 0:DIM + 32]
            nc.tensor.matmul(qps[:, 0:32], lhsT=xTq16[:, qsl], rhs=wqa,
                             start=True, stop=True)
            nc.tensor.matmul(qps[:, 32:32 + DIM], lhsT=posTq16[:, qsl],
                             rhs=pw1a, start=True, stop=True)
            nc.scalar.copy(qp16, qps)

            hid = apool.tile([DIM, NQ], F16, tag="hid")
            h2st = spool.tile([64, NQ // 2], F16, tag="h2st")
            we = apool.tile([DIM, 2 * NQ], F16, tag="we", bufs=3)
            e = we[:, NQ:2 * NQ]
            vpe = apool.tile([DIM, NQ], F16, tag="sp", bufs=3)
            for gc in range(4):
                ssl = slice(gc * 512, (gc + 1) * 512)
                sps = psB.tile([DIM, 512], F32, tag="mm", name="sps")
                nc.tensor.matmul(sps, lhsT=p1qM, rhs=s16[:, ssl],
                                 start=True, stop=False)
                nc.tensor.matmul(sps, lhsT=id16n, rhs=g[:, 2, ssl],
                                 start=False, stop=True)
                nc.scalar.activation(hid[:, ssl], sps, AF.Relu)
            for pr in range(2):
                hp = psB.tile([DIM, 512], F32, tag="mm", name="hp")
                for gc in range(2 * pr, 2 * pr + 2):
                    ssl = slice(gc * 512, (gc + 1) * 512)
                    hpc = hp[(gc % 2) * 32:(gc % 2) * 32 + 32, :]
                    nc.tensor.matmul(hpc, lhsT=xwqa, rhs=s16[:, ssl],
                                     start=True, stop=False)
                    nc.tensor.matmul(hpc, lhsT=aw1n, rhs=g[:, 0, ssl],
                                     start=False, stop=False)
                    nc.tensor.matmul(hpc, lhsT=wp, rhs=hid[:, ssl],
                                     start=False, stop=True)
                nc.scalar.activation(h2st[:, pr * 512:(pr + 1) * 512],
                                     hp[0:64, :], AF.Relu, bias=bh2x2)
            for gc in range(4):
                ssl = slice(gc * 512, (gc + 1) * 512)
                vp = psB.tile([DIM, 512], F32, tag="mm", name="vp")
                nc.tensor.matmul(vp, lhsT=pw2, rhs=hid[:, ssl],
                                 start=True, stop=False)
                nc.tensor.matmul(vp, lhsT=id16, rhs=g[:, 1, ssl],
                                 start=False, stop=True)
                nc.scalar.activation(vpe[:, ssl], vp, AF.Identity)
            for gc in range(4):
                ssl = slice(gc * 512, (gc + 1) * 512)
                lp = psB.tile([DIM, 512], F32, tag="mm", name="lp")
                h2c = h2st[(gc % 2) * 32:(gc % 2) * 32 + 32,
                           (gc // 2) * 512:(gc // 2) * 512 + 512]
                nc.tensor.matmul(lp,
                                 lhsT=aw2r[(gc % 2) * 32:(gc % 2) * 32 + 32, :],
                                 rhs=h2c, start=True, stop=True)
                nc.scalar.activation(e[:, ssl], lp, AF.Exp)
            ev_tiles[t] = (we, vpe)

        def emit_attn_red(t):
            qsl = slice(t * DIM, (t + 1) * DIM)
            we, vpe = ev_tiles.pop(t)
            del g_tiles[t]
            quad = spool.tile([DIM, 512], F32, tag="quad")
            ws, es = quad[:, 0:128], quad[:, 128:256]
            rec, ot = quad[:, 256:384], quad[:, 384:512]
            nc.vector.tensor_mul(we[:, 0:NQ], we[:, NQ:2 * NQ], vpe)

            fs = apool.tile([DIM, 3584], F16, tag="fold")
            s3 = we.rearrange("p (q k) -> p q k", k=16)
            L1 = fs[:, 0:2048].rearrange("p (q k) -> p q k", k=8)
            nc.vector.tensor_add(L1, s3[:, :, 0:8], s3[:, :, 8:16])
            L1v = fs[:, 0:2048].rearrange("p (q k) -> p q k", k=8)
            L2 = fs[:, 2048:3072].rearrange("p (q k) -> p q k", k=4)
            nc.vector.tensor_add(L2, L1v[:, :, 0:4], L1v[:, :, 4:8])
            L2v = fs[:, 2048:3072].rearrange("p (q k) -> p q k", k=4)
            L3 = fs[:, 3072:3584].rearrange("p (q k) -> p q k", k=2)
            nc.vector.tensor_add(L3, L2v[:, :, 0:2], L2v[:, :, 2:4])
            L3v = fs[:, 3072:3584].rearrange("p (q k) -> p q k", k=2)
            nc.vector.tensor_add(quad[:, 0:256].rearrange("p q -> p q ()"),
                                 L3v[:, :, 0:1], L3v[:, :, 1:2])
            nc.vector.scalar_tensor_tensor(ws, es, bu, ws,
                                           op0=mybir.AluOpType.mult,
                                           op1=mybir.AluOpType.add)
            nc.vector.reciprocal(rec, es)
            nc.vector.tensor_mul(ot, ws, rec)

            nc.sync.dma_start(out_d[:, qsl], ot)

        for w in range(-3, NT + 2):
            with tc.tile_wait_until((t0 + (w + 3) * pwin) / 1000.0):
                if 0 <= w + 3 < NT:
                    emit_cw_prefetch(w + 3)
                if 0 <= w + 2 < NT:
                    emit_sel_scan(w + 2)
                if 0 <= w - 1 < NT:
                    emit_attn_mlp(w - 1)
                if 0 <= w + 1 < NT:
                    llC = lib_load(library_config.ap_gather)
                    emit_translate(w + 1, llC)
                if 0 <= w + 2 < NT:
                    llA = lib_load(library_config.local_scatter)
                    emit_sel_finish(w + 2, llA)
                if 0 <= w < NT:
                    llB = lib_load(library_config.mlp)
                    emit_gather(w, llB)
                if 0 <= w - 2 < NT:
                    emit_attn_red(w - 2)
    return nc



def kd_order(p, leaf=128):
    out = []

    def rec(ids):
        if len(ids) <= leaf:
            out.append(ids)
            return
        ext = p[ids].max(0) - p[ids].min(0)
        ax = int(np.argmax(ext))
        srt = ids[np.argsort(p[ids, ax], kind="stable")]
        h = len(srt) // 2
        rec(srt[:h])
        rec(srt[h:])

    rec(np.arange(len(p)))
    return np.concatenate(out)


def make_in_maps(inputs):
    x, pos = np.asarray(inputs["x"]), np.asarray(inputs["pos"])
    f16 = np.float16
    W = {k: np.asarray(v, np.float32) for k, v in inputs.items()}
    pw1p = np.zeros((4, DIM), np.float32)
    pw1p[:3] = W["pw1"]
    pw1p[3] = -W["pb1"]
    s_hot = np.zeros((DIM, NQ), f16)
    s_hot[np.repeat(np.arange(DIM), 16), np.arange(NQ)] = 1.0
    aw2rep = np.zeros((64, DIM), f16)
    for g_ in range(2):
        aw2rep[32 * g_:32 * g_ + 32] = W["aw2"].astype(f16)
    bh2 = (W["ab1"] + (W["qb"] - W["kb"] + W["pb2"]) @ W["aw1"]).reshape(32, 1)
    shared = {
        "kw16": np.ascontiguousarray(W["kw"].astype(f16)),
        "vw16": np.ascontiguousarray(W["vw"].astype(f16)),
        "pw1_16": np.ascontiguousarray(pw1p.astype(f16)),
        "pw2_16": np.ascontiguousarray(W["pw2"].astype(f16)),
        "wqa16": np.ascontiguousarray((W["qw"] @ W["aw1"]).astype(f16)),
        "aw1n16": np.ascontiguousarray((-W["aw1"]).astype(f16)),
        "wp16": np.ascontiguousarray((W["pw2"] @ W["aw1"]).astype(f16)),
        "aw2rep": np.ascontiguousarray(aw2rep),
        "s16hot": np.ascontiguousarray(s_hot),
        "id16": np.eye(DIM, dtype=f16),
        "id16n": (-np.eye(DIM)).astype(f16),
        "bias_h2x2": np.ascontiguousarray(np.tile(bh2, (2, 1)).astype(np.float32)),
        "bias_u": np.ascontiguousarray((W["vb"] + W["pb2"]).reshape(DIM, 1)),
        "offs": np.broadcast_to((np.arange(NSLOT, dtype=np.uint32) // 8) * 128,
                                (DIM, NSLOT)).copy(),
        "ranks": np.broadcast_to(np.arange(1, 17, dtype=np.int16),
                                 (DIM, 16)).copy(),
        "pw1n4_16": np.ascontiguousarray(np.vstack([-W["pw1"],
                                                    np.zeros((1, DIM))]).astype(f16)),
        "id32n128": np.ascontiguousarray(
            np.vstack([np.zeros((32, 32)), -np.eye(32),
                       np.zeros((64, 32))]).astype(f16)),
        "idp128_16": np.eye(4, DIM, dtype=f16),
        "kwa32_16": np.ascontiguousarray((W["kw"] @ W["aw1"]).astype(f16)),
    }
    orders = []
    in_maps = []
    for b in range(B):
        orders.append(kd_order(pos[b].astype(np.float64)))
    for c in range(8):
        b, h = c // 2, c % 2
        order = orders[b]
        ps = pos[b].astype(np.float64)[order]
        xs = np.asarray(x[b], np.float32)[order]
        qs = slice(h * NQ, (h + 1) * NQ)

        p32 = ps.astype(np.float32)
        caug = np.zeros((N, 4), np.float32)
        caug[:, :3] = 2.0 * p32
        caug[:, 3] = (p32[:, 0] * p32[:, 0] + p32[:, 1] * p32[:, 1]) \
            + p32[:, 2] * p32[:, 2]
        qaug = np.zeros((N, 4), np.float32)
        qaug[:, :3] = p32
        qaug[:, 3] = -1.0
        post = np.zeros((4, N), f16)
        post[:3] = p32.T.astype(f16)
        postq = np.zeros((4, NQ), f16)
        postq[:3] = p32[qs].T.astype(f16)
        postq[3] = -1.0

        caugW = np.zeros((4, NT * WCAND), np.float32)
        candg = np.zeros((1, NT * WCAND), np.int32)
        rng = np.random.default_rng(97 + c)
        for t in range(NT):
            rows = slice(h * NQ + t * DIM, h * NQ + (t + 1) * DIM)
            tmin, tmax = ps[rows].min(0), ps[rows].max(0)
            gap = np.maximum(0, np.maximum(tmin[None] - ps, ps - tmax[None]))
            cand = np.argsort((gap ** 2).sum(-1), kind="stable")[:WCAND]
            cand = cand[rng.permutation(WCAND)]
            caugW[:, t * WCAND:(t + 1) * WCAND] = caug[cand].T
            candg[0, t * WCAND:(t + 1) * WCAND] = cand
        m = dict(shared)
        m["xT16"] = np.ascontiguousarray(xs.T.astype(f16))
        m["xTq16"] = np.ascontiguousarray(xs[qs].T.astype(f16))
        m["posT16r"] = np.ascontiguousarray(post)
        m["posTq16r"] = np.ascontiguousarray(postq)
        m["qaugR"] = np.ascontiguousarray(qaug[qs].T)
        m["caugW"] = caugW
        m["candg"] = candg
        in_maps.append(m)
    return in_maps, orders


_CACHED = {}


def run(inputs, trace=False, **spmd_kwargs):
    from concourse.bass_utils import run_bass_kernel_spmd

    if "nc" not in _CACHED:
        import concourse.bacc as bacc
        nc = bacc.Bacc("TRN2", target_bir_lowering=False, debug=False,
                       num_devices=8)
        build(nc)
        nc.compile()
        _CACHED["nc"] = nc
    nc = _CACHED["nc"]
    in_maps, orders = make_in_maps(inputs)
    res = run_bass_kernel_spmd(nc, in_maps, core_ids=list(range(8)),
                               trace=trace, **spmd_kwargs)
    out = np.empty((B, N, DIM), np.float32)
    for c in range(8):
        b, h = c // 2, c % 2
        rows = orders[b][h * NQ:(h + 1) * NQ]
        out[b, rows] = res.results[c]["out"].T
    return out, res


def kernel(**inputs):
    return run(inputs)[0]


# revision 48
# speedup vs baseline: 1.0599x; 1.0346x over previous
import numpy as np
from contextlib import ExitStack

import concourse.bass as bass
import concourse.mybir as mybir
from concourse import library_config
from concourse.tile import TileContext
from concourse.tile_rust import add_dep_helper

F32 = mybir.dt.float32
F16 = mybir.dt.float16
I16 = mybir.dt.int16
I32 = mybir.dt.int32
U32 = mybir.dt.uint32
AF = mybir.ActivationFunctionType

B, N, DIM, K = 4, 4096, 128, 16
NQ = 2048
NT = 16
WCAND = 896
NSEG = WCAND // 128
NSLOT = NSEG * 8
NEG = -3.0e38


def bcast16(ap):
    return ap.rearrange("p q -> p q ()").to_broadcast(list(ap.shape) + [16])


def build(nc, stage="full", pwin=0.001, t0=12.0):
    xT_d = nc.dram_tensor("xT16", [DIM, N], F16, kind="ExternalInput")
    xTq_d = nc.dram_tensor("xTq16", [DIM, NQ], F16, kind="ExternalInput")
    posT_d = nc.dram_tensor("posT16r", [4, N], F16, kind="ExternalInput")
    posTq_d = nc.dram_tensor("posTq16r", [4, NQ], F16, kind="ExternalInput")
    qaugR_d = nc.dram_tensor("qaugR", [4, NQ], F32, kind="ExternalInput")
    caugW_d = nc.dram_tensor("caugW", [4, NT * WCAND], F32, kind="ExternalInput")
    candg_d = nc.dram_tensor("candg", [1, NT * WCAND], I32, kind="ExternalInput")
    kw_d = nc.dram_tensor("kw16", [DIM, DIM], F16, kind="ExternalInput")
    vw_d = nc.dram_tensor("vw16", [DIM, DIM], F16, kind="ExternalInput")
    pw1_d = nc.dram_tensor("pw1_16", [4, DIM], F16, kind="ExternalInput")
    pw2_d = nc.dram_tensor("pw2_16", [DIM, DIM], F16, kind="ExternalInput")
    wqa_d = nc.dram_tensor("wqa16", [DIM, 32], F16, kind="ExternalInput")
    aw1n_d = nc.dram_tensor("aw1n16", [DIM, 32], F16, kind="ExternalInput")
    wp_d = nc.dram_tensor("wp16", [DIM, 32], F16, kind="ExternalInput")
    aw2r_d = nc.dram_tensor("aw2rep", [64, DIM], F16, kind="ExternalInput")
    s16_d = nc.dram_tensor("s16hot", [DIM, NQ], F16, kind="ExternalInput")
    id16_d = nc.dram_tensor("id16", [DIM, DIM], F16, kind="ExternalInput")
    id16n_d = nc.dram_tensor("id16n", [DIM, DIM], F16, kind="ExternalInput")
    bh2_d = nc.dram_tensor("bias_h2x2", [64, 1], F32, kind="ExternalInput")
    bu_d = nc.dram_tensor("bias_u", [DIM, 1], F32, kind="ExternalInput")
    offs_d = nc.dram_tensor("offs", [DIM, NSLOT], U32, kind="ExternalInput")
    ranks_d = nc.dram_tensor("ranks", [DIM, 16], I16, kind="ExternalInput")
    pw1n4_d = nc.dram_tensor("pw1n4_16", [4, DIM], F16, kind="ExternalInput")
    id32n_d = nc.dram_tensor("id32n128", [DIM, 32], F16, kind="ExternalInput")
    idp_d = nc.dram_tensor("idp128_16", [4, DIM], F16, kind="ExternalInput")
    kwa_d = nc.dram_tensor("kwa32_16", [DIM, 32], F16, kind="ExternalInput")

    tbl_d = nc.dram_tensor("tbl", [N, 2 * DIM], F16, kind="Internal")
    itmp_d = nc.dram_tensor("itmp", [NT, DIM, 16], I16, kind="Internal")
    itmp2_d = nc.dram_tensor("itmp2", [NT, 1, 2 * 2048], I16, kind="Internal")
    out_d = nc.dram_tensor("out", [DIM, NQ], F32, kind="ExternalOutput")

    with TileContext(nc) as tc, ExitStack() as ctx:
        const = ctx.enter_context(tc.tile_pool(name="const", bufs=1))
        work = ctx.enter_context(tc.tile_pool(name="work", bufs=2))
        gpool = ctx.enter_context(tc.tile_pool(name="gpool", bufs=3))
        apool = ctx.enter_context(tc.tile_pool(name="apool", bufs=2))
        spool = ctx.enter_context(tc.tile_pool(name="spool", bufs=2))
        ipool = ctx.enter_context(tc.tile_pool(name="ipool", bufs=4))
        cwpool = ctx.enter_context(tc.tile_pool(name="cwpool", bufs=3))
        agpool = ctx.enter_context(tc.tile_pool(name="agpool", bufs=2))
        psA = ctx.enter_context(tc.tile_pool(name="psA", bufs=4, space="PSUM"))
        psB = ctx.enter_context(tc.tile_pool(name="psB", bufs=4, space="PSUM"))

        def cload(d, shape, dtype):
            t = const.tile(shape, dtype, tag=d.name)
            nc.sync.dma_start(t, d[:, :])
            return t

        xT16 = cload(xT_d, [DIM, N], F16)
        kw = cload(kw_d, [DIM, DIM], F16)
        vw = cload(vw_d, [DIM, DIM], F16)
        pw2 = cload(pw2_d, [DIM, DIM], F16)
        wqa = cload(wqa_d, [DIM, 32], F16)
        aw1n = cload(aw1n_d, [DIM, 32], F16)
        wp = cload(wp_d, [DIM, 32], F16)
        aw2r = cload(aw2r_d, [64, DIM], F16)
        s16 = cload(s16_d, [DIM, NQ], F16)
        id16 = cload(id16_d, [DIM, DIM], F16)
        id16n = cload(id16n_d, [DIM, DIM], F16)
        bh2x2 = cload(bh2_d, [64, 1], F32)
        bu = cload(bu_d, [DIM, 1], F32)
        offs = cload(offs_d, [DIM, NSLOT], U32)
        ranks = cload(ranks_d, [DIM, 16], I16)
        pw1n4 = cload(pw1n4_d, [4, DIM], F16)
        id32n128 = cload(id32n_d, [DIM, 32], F16)
        idp128 = cload(idp_d, [4, DIM], F16)
        kwa32 = cload(kwa_d, [DIM, 32], F16)
        xTq16 = cload(xTq_d, [DIM, NQ], F16)

        arena = const.tile([DIM, 3136], F32, tag="arena")
        pw1a = arena[0:4, 0:64].bitcast(F16)
        posTq16 = arena[0:4, 64:1088].bitcast(F16)
        posT16 = arena[0:4, 1088:3136].bitcast(F16)
        nc.sync.dma_start(pw1a, pw1_d[:, :])
        nc.sync.dma_start(posTq16, posTq_d[:, :])
        nc.sync.dma_start(posT16, posT_d[:, :])
        q_aug = const.tile([4, NQ], F32, tag="qaugt")
        nc.sync.dma_start(q_aug, qaugR_d[:, :])

        cgtabA = const.tile([16, WCAND], I32, tag="cgtabA")
        cgtabB = const.tile([16, WCAND], I32, tag="cgtabB")
        cgtabs = [cgtabA, cgtabB]
        nc.gpsimd.memset(cgtabA, 0)
        nc.gpsimd.memset(cgtabB, 0)

        for pblk in range(N // DIM // 2):
            ps = psB.tile([DIM, 512], F32, tag="mm", name="ps")
            for h in range(2):
                bsl = slice((2 * pblk + h) * DIM, (2 * pblk + h + 1) * DIM)
                off = h * 256
                nc.tensor.matmul(ps[:, off:off + 128], lhsT=posT16[:, bsl],
                                 rhs=idp128, start=True, stop=True)
                nc.tensor.matmul(ps[:, off + 32:off + 64], lhsT=xT16[:, bsl],
                                 rhs=kwa32, start=True, stop=True)
                nc.tensor.matmul(ps[:, off + 128:off + 256],
                                 lhsT=xT16[:, bsl], rhs=vw,
                                 start=True, stop=True)
            tb = work.tile([DIM, 512], F16, tag="ldx")
            nc.scalar.copy(tb, ps)
            for h in range(2):
                bsl = slice((2 * pblk + h) * DIM, (2 * pblk + h + 1) * DIM)
                nc.sync.dma_start(tbl_d[bsl, :],
                                  tb[:, h * 256:(h + 1) * 256])

        if stage == "table":
            return nc

        idxall = const.tile([DIM, NT * DIM], I16, tag="idxall")
        pending_pool = []

        def lib_load(lib):
            ll = nc.gpsimd.load_library(lib)
            for op in pending_pool:
                add_dep_helper(ll.ins, op.ins, reason="lib window order")
            del pending_pool[:]
            return ll

        cw_tiles, sel_tiles, idxl_tiles = {}, {}, {}

        def emit_cw_prefetch(t):
            cw = cwpool.tile([4, WCAND], F32, tag="cw")
            nc.sync.dma_start(cw, caugW_d[:, t * WCAND:(t + 1) * WCAND])
            cw_tiles[t] = cw

        def emit_sel_scan(t):
            qsl = slice(t * DIM, (t + 1) * DIM)
            cw = cw_tiles.pop(t)
            v8s = ipool.tile([DIM, NSLOT], F32, tag="v8s")
            p8s = ipool.tile([DIM, NSLOT], U32, tag="p8s")
            ch = 0
            for bk, wd in enumerate([512, WCAND - 512]):
                dps = psA.tile([DIM, 512], F32, tag="dps", name="dps")
                dps = dps[:, 0:wd]
                nc.tensor.matmul(dps, lhsT=q_aug[:, qsl],
                                 rhs=cw[:, bk * 512:bk * 512 + wd],
                                 start=True, stop=True)
                for sg in range(wd // 128):
                    s8 = slice(ch * 8, (ch + 1) * 8)
                    seg = dps[:, sg * 128:(sg + 1) * 128]
                    nc.vector.max(out=v8s[:, s8], in_=seg)
                    nc.vector.max_index(out=p8s[:, s8], in_max=v8s[:, s8],
                                        in_values=seg)
                    ch += 1
            posg16 = ipool.tile([DIM, NSLOT], I16, tag="posg16")
            nc.vector.tensor_add(posg16, p8s, offs)

            mm8 = ipool.tile([DIM, 16], F32, tag="mm8")
            m8a, m8b = mm8[:, 0:8], mm8[:, 8:16]
            sl16 = ipool.tile([DIM, 16], U32, tag="sl16")
            nc.vector.max(out=m8a, in_=v8s)
            nc.vector.max_index(out=sl16[:, 0:8], in_max=m8a, in_values=v8s)
            nc.vector.match_replace(out=v8s, in_to_replace=m8a, in_values=v8s,
                                    imm_value=NEG)
            nc.vector.max(out=m8b, in_=v8s)
            nc.vector.max_index(out=sl16[:, 8:16], in_max=m8b, in_values=v8s)
            slots16 = ipool.tile([DIM, 16], I16, tag="slots16")
            nc.vector.tensor_copy(slots16, sl16)
            sel_tiles[t] = (slots16, posg16)

        def emit_sel_finish(t, llA):
            slots16, posg16 = sel_tiles.pop(t)
            R = ipool.tile([DIM, NSLOT], I16, tag="R")
            ls1 = nc.gpsimd.local_scatter(R[:, :], ranks[:, :], slots16[:, :],
                                          channels=DIM, num_elems=NSLOT,
                                          num_idxs=16)
            add_dep_helper(ls1.ins, llA.ins, reason="needs local_scatter lib")
            pending_pool.append(ls1)
            nc.vector.tensor_scalar(R, R, 1, None,
                                    op0=mybir.AluOpType.subtract)
            idxsel = ipool.tile([DIM, 16], I16, tag="idxsel")
            ls2 = nc.gpsimd.local_scatter(idxsel[:, :], posg16[:, :], R[:, :],
                                          channels=DIM, num_elems=16,
                                          num_idxs=NSLOT)
            add_dep_helper(ls2.ins, llA.ins, reason="needs local_scatter lib")
            pending_pool.append(ls2)

            nc.sync.dma_start(itmp_d[t, :, :], idxsel)
            idxl = ipool.tile([16, DIM], I16, tag="idxl")
            nc.sync.dma_start(idxl, itmp_d[t, :, :].rearrange("a b -> b a"))
            idxl_tiles[t] = idxl
            nc.sync.dma_start(cgtabs[t % 2][0:1, :],
                              candg_d[:, t * WCAND:(t + 1) * WCAND])

        def emit_translate(t, llC):
            isl = slice(t * DIM, (t + 1) * DIM)
            idxl = idxl_tiles.pop(t)
            ag = agpool.tile([16, 2048], I32, tag="ag")
            gi = nc.gpsimd.ap_gather(out_ap=ag[:, :],
                                     in_ap=cgtabs[t % 2][:, :],
                                     idxs_ap=idxl[:, :], channels=16,
                                     num_elems=WCAND, d=1, num_idxs=2048)
            add_dep_helper(gi.ins, llC.ins, reason="needs ap_gather lib")
            pending_pool.append(gi)
            nc.sync.dma_start(itmp2_d[t, :, :], ag[0:1, :].bitcast(I16))
            src = itmp2_d[t, 0, :].rearrange("(q k two) -> k q two",
                                             k=16, two=2)[:, :, 0:1]
            nc.sync.dma_start(
                idxall[0:16, isl].rearrange("k q -> k q ()"), src)
            nc.sync.dma_start(idxall[16:32, isl], idxall[0:16, isl])
            nc.sync.dma_start(idxall[32:64, isl], idxall[0:32, isl])
            nc.sync.dma_start(idxall[64:128, isl], idxall[0:64, isl])

        g_tiles, ev_tiles = {}, {}

        def emit_gather(t, llB):
            ibase = t * DIM
            g = gpool.tile([DIM, 3, 4, 512], F16, tag="g")
            g_tiles[t] = g
            for gc in range(4):
                gi = nc.gpsimd.dma_gather(
                    out_ap=g[:, :, gc, :],
                    in_ap=tbl_d[:, :],
                    idxs_ap=idxall[:, ibase + gc * 32:ibase + (gc + 1) * 32],
                    num_idxs=512,
                    num_idxs_reg=512,
                    elem_size=2 * DIM,
                    transpose=True,
                )
                add_dep_helper(gi.ins, llB.ins, reason="needs mlp lib")
                pending_pool.append(gi)

        def emit_attn_mlp(t):
            qsl = slice(t * DIM, (t + 1) * DIM)
            g = g_tiles[t]

            qp16 = spool.tile([DIM, DIM + 32], F16, tag="qp16")
            xwqa, p1qM = qp16[:, 0:32], qp16[:, 32:DIM + 32]
            qps = psB.tile([DIM, 512], F32, tag="mm", name="qps")
            qps = qps[:, 0:DIM + 32]
            nc.tensor.matmul(qps[:, 0:32], lhsT=xTq16[:, qsl], rhs=wqa,
                             start=True, stop=True)
            nc.tensor.matmul(qps[:, 32:32 + DIM], lhsT=posTq16[:, qsl],
                             rhs=pw1a, start=True, stop=True)
            nc.scalar.copy(qp16, qps)

            hid = apool.tile([DIM, NQ], F16, tag="hid")
            h2st = spool.tile([64, NQ // 2], F16, tag="h2st")
            we = apool.tile([DIM, 2 * NQ], F16, tag="we", bufs=3)
            e = we[:, NQ:2 * NQ]
            vpe = apool.tile([DIM, NQ], F16, tag="sp", bufs=3)
            for gc in range(4):
                ssl = slice(gc * 512, (gc + 1) * 512)
                sps = psB.tile([DIM, 512], F32, tag="mm", name="sps")
                nc.tensor.matmul(sps, lhsT=p1qM, rhs=s16[:, ssl],
                                 start=True, stop=False)
                nc.tensor.matmul(sps, lhsT=id16n, rhs=g[:, 2, ssl],
                                 start=False, stop=True)
                nc.scalar.activation(hid[:, ssl], sps, AF.Relu)
            for pr in range(2):
                hp = psB.tile([DIM, 512], F32, tag="mm", name="hp")
                for gc in range(2 * pr, 2 * pr + 2):
                    ssl = slice(gc * 512, (gc + 1) * 512)
                    hpc = hp[(gc % 2) * 32:(gc % 2) * 32 + 32, :]
                    nc.tensor.matmul(hpc, lhsT=xwqa, rhs=s16[:, ssl],
                                     start=True, stop=False)
                    nc.tensor.matmul(hpc, lhsT=aw1n, rhs=g[:, 0, ssl],
                                     start=False, stop=False)
                    nc.tensor.matmul(hpc, lhsT=wp, rhs=hid[:, ssl],
                                     start=False, stop=True)
                nc.scalar.activation(h2st[:, pr * 512:(pr + 1) * 512],
                                     hp[0:64, :], AF.Relu, bias=bh2x2)
            for gc in range(4):
                ssl = slice(gc * 512, (gc + 1) * 512)
                vp = psB.tile([DIM, 512], F32, tag="mm", name="vp")
                nc.tensor.matmul(vp, lhsT=pw2, rhs=hid[:, ssl],
                                 start=True, stop=False)
                nc.tensor.matmul(vp, lhsT=id16, rhs=g[:, 1, ssl],
                                 start=False, stop=True)
                nc.scalar.activation(vpe[:, ssl], vp, AF.Identity)
            for gc in range(4):
                ssl = slice(gc * 512, (gc + 1) * 512)
                lp = psB.tile([DIM, 512], F32, tag="mm", name="lp")
                h2c = h2st[(gc % 2) * 32:(gc % 2) * 32 + 32,
                           (gc // 2) * 512:(gc // 2) * 512 + 512]
                nc.tensor.matmul(lp,
                                 lhsT=aw2r[(gc % 2) * 32:(gc % 2) * 32 + 32, :],
                                 rhs=h2c, start=True, stop=True)
                nc.scalar.activation(e[:, ssl], lp, AF.Exp)
            ev_tiles[t] = (we, vpe)

        def emit_attn_red(t):
            qsl = slice(t * DIM, (t + 1) * DIM)
            we, vpe = ev_tiles.pop(t)
            del g_tiles[t]
            quad = spool.tile([DIM, 512], F32, tag="quad")
            ws, es = quad[:, 0:128], quad[:, 128:256]
            rec, ot = quad[:, 256:384], quad[:, 384:512]
            nc.vector.tensor_mul(we[:, 0:NQ], we[:, NQ:2 * NQ], vpe)

            fs = apool.tile([DIM, 3584], F16, tag="fold")
            s3 = we.rearrange("p (q k) -> p q k", k=16)
            L1 = fs[:, 0:2048].rearrange("p (q k) -> p q k", k=8)
            nc.vector.tensor_add(L1, s3[:, :, 0:8], s3[:, :, 8:16])
            L1v = fs[:, 0:2048].rearrange("p (q k) -> p q k", k=8)
            L2 = fs[:, 2048:3072].rearrange("p (q k) -> p q k", k=4)
            nc.vector.tensor_add(L2, L1v[:, :, 0:4], L1v[:, :, 4:8])
            L2v = fs[:, 2048:3072].rearrange("p (q k) -> p q k", k=4)
            L3 = fs[:, 3072:3584].rearrange("p (q k) -> p q k", k=2)
            nc.vector.tensor_add(L3, L2v[:, :, 0:2], L2v[:, :, 2:4])
            L3v = fs[:, 3072:3584].rearrange("p (q k) -> p q k", k=2)
            nc.vector.tensor_add(quad[:, 0:256].rearrange("p q -> p q ()"),
                                 L3v[:, :, 0:1], L3v[:, :, 1:2])
            nc.vector.scalar_tensor_tensor(ws, es, bu, ws,
                                           op0=mybir.AluOpType.mult,
                                           op1=mybir.AluOpType.add)
            nc.vector.reciprocal(rec, es)
            nc.vector.tensor_mul(ot, ws, rec)

            nc.sync.dma_start(out_d[:, qsl], ot)

        for w in range(-3, NT + 2):
            with tc.tile_wait_until((t0 + (w + 3) * pwin) / 1000.0):
                if 0 <= w + 3 < NT:
                    emit_cw_prefetch(w + 3)
                if 0 <= w + 2 < NT:
                    emit_sel_scan(w + 2)
                if 0 <= w - 1 < NT:
                    emit_attn_mlp(w - 1)
                if 0 <= w + 1 < NT:
                    llC = lib_load(library_config.ap_gather)
                    emit_translate(w + 1, llC)
                if 0 <= w + 2 < NT:
                    llA = lib_load(library_config.local_scatter)
                    emit_sel_finish(w + 2, llA)
                if 0 <= w < NT:
                    llB = lib_load(library_config.mlp)
                    emit_gather(w, llB)
                if 0 <= w - 2 < NT:
                    emit_attn_red(w - 2)
    return nc



def kd_order(p, leaf=128):
    out = []

    def rec(ids):
        if len(ids) <= leaf:
            out.append(ids)
            return
        ext = p[ids].max(0) - p[ids].min(0)
        ax = int(np.argmax(ext))
        srt = ids[np.argsort(p[ids, ax], kind="stable")]
        h = len(srt) // 2
        rec(srt[:h])
        rec(srt[h:])

    rec(np.arange(len(p)))
    return np.concatenate(out)


def make_in_maps(inputs):
    x, pos = np.asarray(inputs["x"]), np.asarray(inputs["pos"])
    f16 = np.float16
    W = {k: np.asarray(v, np.float32) for k, v in inputs.items()}
    pw1p = np.zeros((4, DIM), np.float32)
    pw1p[:3] = W["pw1"]
    pw1p[3] = -W["pb1"]
    s_hot = np.zeros((DIM, NQ), f16)
    s_hot[np.repeat(np.arange(DIM), 16), np.arange(NQ)] = 1.0
    aw2rep = np.zeros((64, DIM), f16)
    for g_ in range(2):
        aw2rep[32 * g_:32 * g_ + 32] = W["aw2"].astype(f16)
    bh2 = (W["ab1"] + (W["qb"] - W["kb"] + W["pb2"]) @ W["aw1"]).reshape(32, 1)
    shared = {
        "kw16": np.ascontiguousarray(W["kw"].astype(f16)),
        "vw16": np.ascontiguousarray(W["vw"].astype(f16)),
        "pw1_16": np.ascontiguousarray(pw1p.astype(f16)),
        "pw2_16": np.ascontiguousarray(W["pw2"].astype(f16)),
        "wqa16": np.ascontiguousarray((W["qw"] @ W["aw1"]).astype(f16)),
        "aw1n16": np.ascontiguousarray((-W["aw1"]).astype(f16)),
        "wp16": np.ascontiguousarray((W["pw2"] @ W["aw1"]).astype(f16)),
        "aw2rep": np.ascontiguousarray(aw2rep),
        "s16hot": np.ascontiguousarray(s_hot),
        "id16": np.eye(DIM, dtype=f16),
        "id16n": (-np.eye(DIM)).astype(f16),
        "bias_h2x2": np.ascontiguousarray(np.tile(bh2, (2, 1)).astype(np.float32)),
        "bias_u": np.ascontiguousarray((W["vb"] + W["pb2"]).reshape(DIM, 1)),
        "offs": np.broadcast_to((np.arange(NSLOT, dtype=np.uint32) // 8) * 128,
                                (DIM, NSLOT)).copy(),
        "ranks": np.broadcast_to(np.arange(1, 17, dtype=np.int16),
                                 (DIM, 16)).copy(),
        "pw1n4_16": np.ascontiguousarray(np.vstack([-W["pw1"],
                                                    np.zeros((1, DIM))]).astype(f16)),
        "id32n128": np.ascontiguousarray(
            np.vstack([np.zeros((32, 32)), -np.eye(32),
                       np.zeros((64, 32))]).astype(f16)),
        "idp128_16": np.eye(4, DIM, dtype=f16),
        "kwa32_16": np.ascontiguousarray((W["kw"] @ W["aw1"]).astype(f16)),
    }
    orders = []
    in_maps = []
    for b in range(B):
        orders.append(kd_order(pos[b].astype(np.float64)))
    for c in range(8):
        b, h = c // 2, c % 2
        order = orders[b]
        ps = pos[b].astype(np.float64)[order]
        xs = np.asarray(x[b], np.float32)[order]
        qs = slice(h * NQ, (h + 1) * NQ)

        p32 = ps.astype(np.float32)
        caug = np.zeros((N, 4), np.float32)
        caug[:, :3] = 2.0 * p32
        caug[:, 3] = (p32[:, 0] * p32[:, 0] + p32[:, 1] * p32[:, 1]) \
            + p32[:, 2] * p32[:, 2]
        qaug = np.zeros((N, 4), np.float32)
        qaug[:, :3] = p32
        qaug[:, 3] = -1.0
        post = np.zeros((4, N), f16)
        post[:3] = p32.T.astype(f16)
        postq = np.zeros((4, NQ), f16)
        postq[:3] = p32[qs].T.astype(f16)
        postq[3] = -1.0

        caugW = np.zeros((4, NT * WCAND), np.float32)
        candg = np.zeros((1, NT * WCAND), np.int32)
        rng = np.random.default_rng(97 + c)
        for t in range(NT):
            rows = slice(h * NQ + t * DIM, h * NQ + (t + 1) * DIM)
            tmin, tmax = ps[rows].min(0), ps[rows].max(0)
            gap = np.maximum(0, np.maximum(tmin[None] - ps, ps - tmax[None]))
            cand = np.argsort((gap ** 2).sum(-1), kind="stable")[:WCAND]
            cand = cand[rng.permutation(WCAND)]
            caugW[:, t * WCAND:(t + 1) * WCAND] = caug[cand].T
            candg[0, t * WCAND:(t + 1) * WCAND] = cand
        m = dict(shared)
        m["xT16"] = np.ascontiguousarray(xs.T.astype(f16))
        m["xTq16"] = np.ascontiguousarray(xs[qs].T.astype(f16))
        m["posT16r"] = np.ascontiguousarray(post)
        m["posTq16r"] = np.ascontiguousarray(postq)
        m["qaugR"] = np.ascontiguousarray(qaug[qs].T)
        m["caugW"] = caugW
        m["candg"] = candg
        in_maps.append(m)
    return in_maps, orders


_CACHED = {}


def run(inputs, trace=False, **spmd_kwargs):
    from concourse.bass_utils import run_bass_kernel_spmd

    if "nc" not in _CACHED:
        import concourse.bacc as bacc
        nc = bacc.Bacc("TRN2", target_bir_lowering=False, debug=False,
                       num_devices=8)
        build(nc)
        nc.compile()
        _CACHED["nc"] = nc
    nc = _CACHED["nc"]
    in_maps, orders = make_in_maps(inputs)
    res = run_bass_kernel_spmd(nc, in_maps, core_ids=list(range(8)),
                               trace=trace, **spmd_kwargs)
    out = np.empty((B, N, DIM), np.float32)
    for c in range(8):
        b, h = c // 2, c % 2
        rows = orders[b][h * NQ:(h + 1) * NQ]
        out[b, rows] = res.results[c]["out"].T
    return out, res


def kernel(**inputs):
    return run(inputs)[0]


# revision 49
# speedup vs baseline: 1.0788x; 1.0179x over previous
import numpy as np
from contextlib import ExitStack

import concourse.bass as bass
import concourse.mybir as mybir
from concourse import library_config
from concourse.tile import TileContext
from concourse.tile_rust import add_dep_helper

F32 = mybir.dt.float32
F16 = mybir.dt.float16
I16 = mybir.dt.int16
I32 = mybir.dt.int32
U32 = mybir.dt.uint32
AF = mybir.ActivationFunctionType

B, N, DIM, K = 4, 4096, 128, 16
NQ = 2048
NT = 16
WCAND = 896
NSEG = WCAND // 128
NSLOT = NSEG * 8
NEG = -3.0e38


def bcast16(ap):
    return ap.rearrange("p q -> p q ()").to_broadcast(list(ap.shape) + [16])


def build(nc, stage="full", pwin=0.001, t0=12.0):
    xT_d = nc.dram_tensor("xT16", [DIM, N], F16, kind="ExternalInput")
    xTq_d = nc.dram_tensor("xTq16", [DIM, NQ], F16, kind="ExternalInput")
    posT_d = nc.dram_tensor("posT16r", [4, N], F16, kind="ExternalInput")
    posTq_d = nc.dram_tensor("posTq16r", [4, NQ], F16, kind="ExternalInput")
    qaugR_d = nc.dram_tensor("qaugR", [4, NQ], F32, kind="ExternalInput")
    caugW_d = nc.dram_tensor("caugW", [4, NT * WCAND], F32, kind="ExternalInput")
    candg_d = nc.dram_tensor("candg", [1, NT * WCAND], I32, kind="ExternalInput")
    kw_d = nc.dram_tensor("kw16", [DIM, DIM], F16, kind="ExternalInput")
    vw_d = nc.dram_tensor("vw16", [DIM, DIM], F16, kind="ExternalInput")
    pw1_d = nc.dram_tensor("pw1_16", [4, DIM], F16, kind="ExternalInput")
    pw2_d = nc.dram_tensor("pw2_16", [DIM, DIM], F16, kind="ExternalInput")
    wqa_d = nc.dram_tensor("wqa16", [DIM, 32], F16, kind="ExternalInput")
    aw1n_d = nc.dram_tensor("aw1n16", [DIM, 32], F16, kind="ExternalInput")
    wp_d = nc.dram_tensor("wp16", [DIM, 32], F16, kind="ExternalInput")
    aw2r_d = nc.dram_tensor("aw2rep", [64, DIM], F16, kind="ExternalInput")
    s16_d = nc.dram_tensor("s16hot", [DIM, NQ], F16, kind="ExternalInput")
    id16_d = nc.dram_tensor("id16", [DIM, DIM], F16, kind="ExternalInput")
    id16n_d = nc.dram_tensor("id16n", [DIM, DIM], F16, kind="ExternalInput")
    bh2_d = nc.dram_tensor("bias_h2x2", [64, 1], F32, kind="ExternalInput")
    bu_d = nc.dram_tensor("bias_u", [DIM, 1], F32, kind="ExternalInput")
    offs_d = nc.dram_tensor("offs", [DIM, NSLOT], U32, kind="ExternalInput")
    ranks_d = nc.dram_tensor("ranks", [DIM, 16], I16, kind="ExternalInput")
    pw1n4_d = nc.dram_tensor("pw1n4_16", [4, DIM], F16, kind="ExternalInput")
    id32n_d = nc.dram_tensor("id32n128", [DIM, 32], F16, kind="ExternalInput")
    idp_d = nc.dram_tensor("idp128_16", [4, DIM], F16, kind="ExternalInput")
    kwa_d = nc.dram_tensor("kwa32_16", [DIM, 32], F16, kind="ExternalInput")

    tbl_d = nc.dram_tensor("tbl", [N, 2 * DIM], F16, kind="Internal")
    itmp_d = nc.dram_tensor("itmp", [NT, DIM, 16], I16, kind="Internal")
    itmp2_d = nc.dram_tensor("itmp2", [NT, 1, 2 * 2048], I16, kind="Internal")
    out_d = nc.dram_tensor("out", [DIM, NQ], F32, kind="ExternalOutput")

    with TileContext(nc) as tc, ExitStack() as ctx:
        const = ctx.enter_context(tc.tile_pool(name="const", bufs=1))
        work = ctx.enter_context(tc.tile_pool(name="work", bufs=2))
        gpool = ctx.enter_context(tc.tile_pool(name="gpool", bufs=3))
        apool = ctx.enter_context(tc.tile_pool(name="apool", bufs=2))
        spool = ctx.enter_context(tc.tile_pool(name="spool", bufs=2))
        ipool = ctx.enter_context(tc.tile_pool(name="ipool", bufs=4))
        cwpool = ctx.enter_context(tc.tile_pool(name="cwpool", bufs=3))
        agpool = ctx.enter_context(tc.tile_pool(name="agpool", bufs=2))
        psA = ctx.enter_context(tc.tile_pool(name="psA", bufs=4, space="PSUM"))
        psB = ctx.enter_context(tc.tile_pool(name="psB", bufs=4, space="PSUM"))

        def cload(d, shape, dtype):
            t = const.tile(shape, dtype, tag=d.name)
            nc.sync.dma_start(t, d[:, :])
            return t

        xT16 = cload(xT_d, [DIM, N], F16)
        vw = cload(vw_d, [DIM, DIM], F16)
        kwa32 = cload(kwa_d, [DIM, 32], F16)
        idp128 = cload(idp_d, [4, DIM], F16)
        offs = cload(offs_d, [DIM, NSLOT], U32)
        ranks = cload(ranks_d, [DIM, 16], I16)
        pw2 = wqa = wp = aw2r = s16 = id16 = bh2x2 = bu = None
        pw1n4 = id32n128 = xTq16 = None

        def load_mlp_consts():
            nonlocal pw2, wqa, wp, aw2r, s16, id16, bh2x2, bu
            nonlocal pw1n4, id32n128, xTq16
            pw2 = cload(pw2_d, [DIM, DIM], F16)
            wqa = cload(wqa_d, [DIM, 32], F16)
            wp = cload(wp_d, [DIM, 32], F16)
            aw2r = cload(aw2r_d, [64, DIM], F16)
            s16 = cload(s16_d, [DIM, NQ], F16)
            id16 = cload(id16_d, [DIM, DIM], F16)
            bh2x2 = cload(bh2_d, [64, 1], F32)
            bu = cload(bu_d, [DIM, 1], F32)
            pw1n4 = cload(pw1n4_d, [4, DIM], F16)
            id32n128 = cload(id32n_d, [DIM, 32], F16)
            xTq16 = cload(xTq_d, [DIM, NQ], F16)

        arena = const.tile([DIM, 3136], F32, tag="arena")
        pw1a = arena[0:4, 0:64].bitcast(F16)
        posTq16 = arena[0:4, 64:1088].bitcast(F16)
        posT16 = arena[0:4, 1088:3136].bitcast(F16)
        nc.sync.dma_start(pw1a, pw1_d[:, :])
        nc.sync.dma_start(posTq16, posTq_d[:, :])
        nc.sync.dma_start(posT16, posT_d[:, :])
        q_aug = const.tile([4, NQ], F32, tag="qaugt")
        nc.sync.dma_start(q_aug, qaugR_d[:, :])

        cgtabA = const.tile([16, WCAND], I32, tag="cgtabA")
        cgtabB = const.tile([16, WCAND], I32, tag="cgtabB")
        cgtabs = [cgtabA, cgtabB]
        nc.gpsimd.memset(cgtabA, 0)
        nc.gpsimd.memset(cgtabB, 0)

        for pblk in range(N // DIM // 2):
            ps = psB.tile([DIM, 512], F32, tag="mm", name="ps")
            for h in range(2):
                bsl = slice((2 * pblk + h) * DIM, (2 * pblk + h + 1) * DIM)
                off = h * 256
                nc.tensor.matmul(ps[:, off:off + 128], lhsT=posT16[:, bsl],
                                 rhs=idp128, start=True, stop=True)
                nc.tensor.matmul(ps[:, off + 32:off + 64], lhsT=xT16[:, bsl],
                                 rhs=kwa32, start=True, stop=True)
                nc.tensor.matmul(ps[:, off + 128:off + 256],
                                 lhsT=xT16[:, bsl], rhs=vw,
                                 start=True, stop=True)
            tb = work.tile([DIM, 512], F16, tag="ldx")
            nc.scalar.copy(tb, ps)
            for h in range(2):
                bsl = slice((2 * pblk + h) * DIM, (2 * pblk + h + 1) * DIM)
                nc.sync.dma_start(tbl_d[bsl, :],
                                  tb[:, h * 256:(h + 1) * 256])

        if stage == "table":
            return nc

        idxall = const.tile([DIM, NT * DIM], I16, tag="idxall")
        pending_pool = []

        def lib_load(lib):
            ll = nc.gpsimd.load_library(lib)
            for op in pending_pool:
                add_dep_helper(ll.ins, op.ins, reason="lib window order")
            del pending_pool[:]
            return ll

        cw_tiles, sel_tiles, idxl_tiles = {}, {}, {}

        def emit_cw_prefetch(t):
            cw = cwpool.tile([4, WCAND], F32, tag="cw")
            nc.sync.dma_start(cw, caugW_d[:, t * WCAND:(t + 1) * WCAND])
            cw_tiles[t] = cw

        def emit_sel_scan(t):
            qsl = slice(t * DIM, (t + 1) * DIM)
            cw = cw_tiles.pop(t)
            v8s = ipool.tile([DIM, NSLOT], F32, tag="v8s")
            p8s = ipool.tile([DIM, NSLOT], U32, tag="p8s")
            ch = 0
            for bk, wd in enumerate([512, WCAND - 512]):
                dps = psA.tile([DIM, 512], F32, tag="dps", name="dps")
                dps = dps[:, 0:wd]
                nc.tensor.matmul(dps, lhsT=q_aug[:, qsl],
                                 rhs=cw[:, bk * 512:bk * 512 + wd],
                                 start=True, stop=True)
                for sg in range(wd // 128):
                    s8 = slice(ch * 8, (ch + 1) * 8)
                    seg = dps[:, sg * 128:(sg + 1) * 128]
                    nc.vector.max(out=v8s[:, s8], in_=seg)
                    nc.vector.max_index(out=p8s[:, s8], in_max=v8s[:, s8],
                                        in_values=seg)
                    ch += 1
            posg16 = ipool.tile([DIM, NSLOT], I16, tag="posg16")
            nc.vector.tensor_add(posg16, p8s, offs)

            mm8 = ipool.tile([DIM, 16], F32, tag="mm8")
            m8a, m8b = mm8[:, 0:8], mm8[:, 8:16]
            sl16 = ipool.tile([DIM, 16], U32, tag="sl16")
            nc.vector.max(out=m8a, in_=v8s)
            nc.vector.max_index(out=sl16[:, 0:8], in_max=m8a, in_values=v8s)
            nc.vector.match_replace(out=v8s, in_to_replace=m8a, in_values=v8s,
                                    imm_value=NEG)
            nc.vector.max(out=m8b, in_=v8s)
            nc.vector.max_index(out=sl16[:, 8:16], in_max=m8b, in_values=v8s)
            slots16 = ipool.tile([DIM, 16], I16, tag="slots16")
            nc.vector.tensor_copy(slots16, sl16)
            sel_tiles[t] = (slots16, posg16)

        def emit_sel_finish(t, llA):
            slots16, posg16 = sel_tiles.pop(t)
            R = ipool.tile([DIM, NSLOT], I16, tag="R")
            ls1 = nc.gpsimd.local_scatter(R[:, :], ranks[:, :], slots16[:, :],
                                          channels=DIM, num_elems=NSLOT,
                                          num_idxs=16)
            add_dep_helper(ls1.ins, llA.ins, reason="needs local_scatter lib")
            pending_pool.append(ls1)
            nc.vector.tensor_scalar(R, R, 1, None,
                                    op0=mybir.AluOpType.subtract)
            idxsel = ipool.tile([DIM, 16], I16, tag="idxsel")
            ls2 = nc.gpsimd.local_scatter(idxsel[:, :], posg16[:, :], R[:, :],
                                          channels=DIM, num_elems=16,
                                          num_idxs=NSLOT)
            add_dep_helper(ls2.ins, llA.ins, reason="needs local_scatter lib")
            pending_pool.append(ls2)

            nc.sync.dma_start(itmp_d[t, :, :], idxsel)
            idxl = ipool.tile([16, DIM], I16, tag="idxl")
            nc.sync.dma_start(idxl, itmp_d[t, :, :].rearrange("a b -> b a"))
            idxl_tiles[t] = idxl
            nc.sync.dma_start(cgtabs[t % 2][0:1, :],
                              candg_d[:, t * WCAND:(t + 1) * WCAND])

        def emit_translate(t, llC):
            isl = slice(t * DIM, (t + 1) * DIM)
            idxl = idxl_tiles.pop(t)
            ag = agpool.tile([16, 2048], I32, tag="ag")
            gi = nc.gpsimd.ap_gather(out_ap=ag[:, :],
                                     in_ap=cgtabs[t % 2][:, :],
                                     idxs_ap=idxl[:, :], channels=16,
                                     num_elems=WCAND, d=1, num_idxs=2048)
            add_dep_helper(gi.ins, llC.ins, reason="needs ap_gather lib")
            pending_pool.append(gi)
            nc.sync.dma_start(itmp2_d[t, :, :], ag[0:1, :].bitcast(I16))
            src = itmp2_d[t, 0, :].rearrange("(q k two) -> k q two",
                                             k=16, two=2)[:, :, 0:1]
            nc.sync.dma_start(
                idxall[0:16, isl].rearrange("k q -> k q ()"), src)
            nc.sync.dma_start(idxall[16:32, isl], idxall[0:16, isl])
            nc.sync.dma_start(idxall[32:64, isl], idxall[0:32, isl])
            nc.sync.dma_start(idxall[64:128, isl], idxall[0:64, isl])

        g_tiles, ev_tiles = {}, {}

        def emit_gather(t, llB):
            ibase = t * DIM
            g = gpool.tile([DIM, 3, 4, 512], F16, tag="g")
            g_tiles[t] = g
            for gc in range(4):
                gi = nc.gpsimd.dma_gather(
                    out_ap=g[:, :, gc, :],
                    in_ap=tbl_d[:, :],
                    idxs_ap=idxall[:, ibase + gc * 32:ibase + (gc + 1) * 32],
                    num_idxs=512,
                    num_idxs_reg=512,
                    elem_size=2 * DIM,
                    transpose=True,
                )
                add_dep_helper(gi.ins, llB.ins, reason="needs mlp lib")
                pending_pool.append(gi)

        def emit_attn_mlp(t):
            qsl = slice(t * DIM, (t + 1) * DIM)
            g = g_tiles[t]

            qp16 = spool.tile([DIM, DIM + 32], F16, tag="qp16")
            xwqa, p1qM = qp16[:, 0:32], qp16[:, 32:DIM + 32]
            qps = psB.tile([DIM, 512], F32, tag="mm", name="qps")
            qps = qps[:, 0:DIM + 32]
            nc.tensor.matmul(qps[:, 0:32], lhsT=xTq16[:, qsl], rhs=wqa,
                             start=True, stop=True)
            nc.tensor.matmul(qps[:, 32:32 + DIM], lhsT=posTq16[:, qsl],
                             rhs=pw1a, start=True, stop=True)
            nc.scalar.copy(qp16, qps)

            hid = apool.tile([DIM, NQ], F16, tag="hid")
            h2st = spool.tile([64, NQ // 2], F16, tag="h2st")
            we = apool.tile([DIM, 2 * NQ], F16, tag="we", bufs=3)
            e = we[:, NQ:2 * NQ]
            vpe = apool.tile([DIM, NQ], F16, tag="sp", bufs=3)
            for gc in range(4):
                ssl = slice(gc * 512, (gc + 1) * 512)
                sps = psB.tile([DIM, 512], F32, tag="mm", name="sps")
                nc.tensor.matmul(sps, lhsT=p1qM, rhs=s16[:, ssl],
                                 start=True, stop=False)
                nc.tensor.matmul(sps, lhsT=id16n, rhs=g[:, 2, ssl],
                                 start=False, stop=True)
                nc.scalar.activation(hid[:, ssl], sps, AF.Relu)
            for pr in range(2):
                hp = psB.tile([DIM, 512], F32, tag="mm", name="hp")
                for gc in range(2 * pr, 2 * pr + 2):
                    ssl = slice(gc * 512, (gc + 1) * 512)
                    hpc = hp[(gc % 2) * 32:(gc % 2) * 32 + 32, :]
                    nc.tensor.matmul(hpc, lhsT=xwqa, rhs=s16[:, ssl],
                                     start=True, stop=False)
                    nc.tensor.matmul(hpc, lhsT=aw1n, rhs=g[:, 0, ssl],
                                     start=False, stop=False)
                    nc.tensor.matmul(hpc, lhsT=wp, rhs=hid[:, ssl],
                                     start=False, stop=True)
                nc.scalar.activation(h2st[:, pr * 512:(pr + 1) * 512],
                                     hp[0:64, :], AF.Relu, bias=bh2x2)
            for gc in range(4):
                ssl = slice(gc * 512, (gc + 1) * 512)
                vp = psB.tile([DIM, 512], F32, tag="mm", name="vp")
                nc.tensor.matmul(vp, lhsT=pw2, rhs=hid[:, ssl],
                                 start=True, stop=False)
                nc.tensor.matmul(vp, lhsT=id16, rhs=g[:, 1, ssl],
                                 start=False, stop=True)
                nc.scalar.activation(vpe[:, ssl], vp, AF.Identity)
            for gc in range(4):
                ssl = slice(gc * 512, (gc + 1) * 512)
                lp = psB.tile([DIM, 512], F32, tag="mm", name="lp")
                h2c = h2st[(gc % 2) * 32:(gc % 2) * 32 + 32,
                           (gc // 2) * 512:(gc // 2) * 512 + 512]
                nc.tensor.matmul(lp,
                                 lhsT=aw2r[(gc % 2) * 32:(gc % 2) * 32 + 32, :],
                                 rhs=h2c, start=True, stop=True)
                nc.scalar.activation(e[:, ssl], lp, AF.Exp)
            ev_tiles[t] = (we, vpe)

        def emit_attn_red(t):
            qsl = slice(t * DIM, (t + 1) * DIM)
            we, vpe = ev_tiles.pop(t)
            del g_tiles[t]
            quad = spool.tile([DIM, 512], F32, tag="quad")
            ws, es = quad[:, 0:128], quad[:, 128:256]
            rec, ot = quad[:, 256:384], quad[:, 384:512]
            nc.vector.tensor_mul(we[:, 0:NQ], we[:, NQ:2 * NQ], vpe)

            fs = apool.tile([DIM, 3584], F16, tag="fold")
            s3 = we.rearrange("p (q k) -> p q k", k=16)
            L1 = fs[:, 0:2048].rearrange("p (q k) -> p q k", k=8)
            nc.vector.tensor_add(L1, s3[:, :, 0:8], s3[:, :, 8:16])
            L1v = fs[:, 0:2048].rearrange("p (q k) -> p q k", k=8)
            L2 = fs[:, 2048:3072].rearrange("p (q k) -> p q k", k=4)
            nc.vector.tensor_add(L2, L1v[:, :, 0:4], L1v[:, :, 4:8])
            L2v = fs[:, 2048:3072].rearrange("p (q k) -> p q k", k=4)
            L3 = fs[:, 3072:3584].rearrange("p (q k) -> p q k", k=2)
            nc.vector.tensor_add(L3, L2v[:, :, 0:2], L2v[:, :, 2:4])
            L3v = fs[:, 3072:3584].rearrange("p (q k) -> p q k", k=2)
            nc.vector.tensor_add(quad[:, 0:256].rearrange("p q -> p q ()"),
                                 L3v[:, :, 0:1], L3v[:, :, 1:2])
            nc.vector.scalar_tensor_tensor(ws, es, bu, ws,
                                           op0=mybir.AluOpType.mult,
                                           op1=mybir.AluOpType.add)
            nc.vector.reciprocal(rec, es)
            nc.vector.tensor_mul(ot, ws, rec)

            nc.sync.dma_start(out_d[:, qsl], ot)

        for w in range(-3, NT + 2):
            with tc.tile_wait_until((t0 + (w + 3) * pwin) / 1000.0):
                if w == -2:
                    load_mlp_consts()
                if 0 <= w + 3 < NT:
                    emit_cw_prefetch(w + 3)
                if 0 <= w + 2 < NT:
                    emit_sel_scan(w + 2)
                if 0 <= w - 1 < NT:
                    emit_attn_mlp(w - 1)
                if 0 <= w + 1 < NT:
                    llC = lib_load(library_config.ap_gather)
                    emit_translate(w + 1, llC)
                if 0 <= w + 2 < NT:
                    llA = lib_load(library_config.local_scatter)
                    emit_sel_finish(w + 2, llA)
                if 0 <= w < NT:
                    llB = lib_load(library_config.mlp)
                    emit_gather(w, llB)
                if 0 <= w - 2 < NT:
                    emit_attn_red(w - 2)
    return nc



def kd_order(p, leaf=128):
    out = []

    def rec(ids):
        if len(ids) <= leaf:
            out.append(ids)
            return
        ext = p[ids].max(0) - p[ids].min(0)
        ax = int(np.argmax(ext))
        srt = ids[np.argsort(p[ids, ax], kind="stable")]
        h = len(srt) // 2
        rec(srt[:h])
        rec(srt[h:])

    rec(np.arange(len(p)))
    return np.concatenate(out)


def make_in_maps(inputs):
    x, pos = np.asarray(inputs["x"]), np.asarray(inputs["pos"])
    f16 = np.float16
    W = {k: np.asarray(v, np.float32) for k, v in inputs.items()}
    pw1p = np.zeros((4, DIM), np.float32)
    pw1p[:3] = W["pw1"]
    pw1p[3] = -W["pb1"]
    s_hot = np.zeros((DIM, NQ), f16)
    s_hot[np.repeat(np.arange(DIM), 16), np.arange(NQ)] = 1.0
    aw2rep = np.zeros((64, DIM), f16)
    for g_ in range(2):
        aw2rep[32 * g_:32 * g_ + 32] = W["aw2"].astype(f16)
    bh2 = (W["ab1"] + (W["qb"] - W["kb"] + W["pb2"]) @ W["aw1"]).reshape(32, 1)
    shared = {
        "kw16": np.ascontiguousarray(W["kw"].astype(f16)),
        "vw16": np.ascontiguousarray(W["vw"].astype(f16)),
        "pw1_16": np.ascontiguousarray(pw1p.astype(f16)),
        "pw2_16": np.ascontiguousarray(W["pw2"].astype(f16)),
        "wqa16": np.ascontiguousarray((W["qw"] @ W["aw1"]).astype(f16)),
        "aw1n16": np.ascontiguousarray((-W["aw1"]).astype(f16)),
        "wp16": np.ascontiguousarray((W["pw2"] @ W["aw1"]).astype(f16)),
        "aw2rep": np.ascontiguousarray(aw2rep),
        "s16hot": np.ascontiguousarray(s_hot),
        "id16": np.eye(DIM, dtype=f16),
        "id16n": (-np.eye(DIM)).astype(f16),
        "bias_h2x2": np.ascontiguousarray(np.tile(bh2, (2, 1)).astype(np.float32)),
        "bias_u": np.ascontiguousarray((W["vb"] + W["pb2"]).reshape(DIM, 1)),
        "offs": np.broadcast_to((np.arange(NSLOT, dtype=np.uint32) // 8) * 128,
                                (DIM, NSLOT)).copy(),
        "ranks": np.broadcast_to(np.arange(1, 17, dtype=np.int16),
                                 (DIM, 16)).copy(),
        "pw1n4_16": np.ascontiguousarray(np.vstack([-W["pw1"],
                                                    np.zeros((1, DIM))]).astype(f16)),
        "id32n128": np.ascontiguousarray(
            np.vstack([np.zeros((32, 32)), -np.eye(32),
                       np.zeros((64, 32))]).astype(f16)),
        "idp128_16": np.eye(4, DIM, dtype=f16),
        "kwa32_16": np.ascontiguousarray((W["kw"] @ W["aw1"]).astype(f16)),
    }
    orders = []
    in_maps = []
    for b in range(B):
        orders.append(kd_order(pos[b].astype(np.float64)))
    for c in range(8):
        b, h = c // 2, c % 2
        order = orders[b]
        ps = pos[b].astype(np.float64)[order]
        xs = np.asarray(x[b], np.float32)[order]
        qs = slice(h * NQ, (h + 1) * NQ)

        p32 = ps.astype(np.float32)
        caug = np.zeros((N, 4), np.float32)
        caug[:, :3] = 2.0 * p32
        caug[:, 3] = (p32[:, 0] * p32[:, 0] + p32[:, 1] * p32[:, 1]) \
            + p32[:, 2] * p32[:, 2]
        qaug = np.zeros((N, 4), np.float32)
        qaug[:, :3] = p32
        qaug[:, 3] = -1.0
        post = np.zeros((4, N), f16)
        post[:3] = p32.T.astype(f16)
        postq = np.zeros((4, NQ), f16)
        postq[:3] = p32[qs].T.astype(f16)
        postq[3] = -1.0

        caugW = np.zeros((4, NT * WCAND), np.float32)
        candg = np.zeros((1, NT * WCAND), np.int32)
        rng = np.random.default_rng(97 + c)
        for t in range(NT):
            rows = slice(h * NQ + t * DIM, h * NQ + (t + 1) * DIM)
            tmin, tmax = ps[rows].min(0), ps[rows].max(0)
            gap = np.maximum(0, np.maximum(tmin[None] - ps, ps - tmax[None]))
            cand = np.argsort((gap ** 2).sum(-1), kind="stable")[:WCAND]
            cand = cand[rng.permutation(WCAND)]
            caugW[:, t * WCAND:(t + 1) * WCAND] = caug[cand].T
            candg[0, t * WCAND:(t + 1) * WCAND] = cand
        m = dict(shared)
        m["xT16"] = np.ascontiguousarray(xs.T.astype(f16))
        m["xTq16"] = np.ascontiguousarray(xs[qs].T.astype(f16))
        m["posT16r"] = np.ascontiguousarray(post)
        m["posTq16r"] = np.ascontiguousarray(postq)
        m["qaugR"] = np.ascontiguousarray(qaug[qs].T)
        m["caugW"] = caugW
        m["candg"] = candg
        in_maps.append(m)
    return in_maps, orders


_CACHED = {}


def run(inputs, trace=False, **spmd_kwargs):
    from concourse.bass_utils import run_bass_kernel_spmd

    if "nc" not in _CACHED:
        import concourse.bacc as bacc
        nc = bacc.Bacc("TRN2", target_bir_lowering=False, debug=False,
                       num_devices=8)
        build(nc)
        nc.compile()
        _CACHED["nc"] = nc
    nc = _CACHED["nc"]
    in_maps, orders = make_in_maps(inputs)
    res = run_bass_kernel_spmd(nc, in_maps, core_ids=list(range(8)),
                               trace=trace, **spmd_kwargs)
    out = np.empty((B, N, DIM), np.float32)
    for c in range(8):
        b, h = c // 2, c % 2
        rows = orders[b][h * NQ:(h + 1) * NQ]
        out[b, rows] = res.results[c]["out"].T
    return out, res


def kernel(**inputs):
    return run(inputs)[0]
